# revision 1
# baseline (speedup 1.0000x reference)
"""MoD (mixture-of-depths) MLP wrapper kernel for Trainium2, 8 NeuronCores.

Sharding: core c handles batch row b = c//2 and the half of that row's
top-K tokens with global selection ranks in [h*1024, (h+1)*1024), h = c%2.
Each core computes the full row's router scores + top-K threshold locally
(no collectives), gathers exactly 1024 token rows by rank via indirect DMA,
runs the FFN in bf16 (fp32 accumulation), and scatters results back into a
zero-filled per-core output buffer.  Host sums the two buffers of each row.
"""

import sys, os

sys.path.insert(0, "/opt/trn_rl_repo")

from contextlib import ExitStack

import numpy as np

from concourse import bass, mybir
from concourse import bacc
import concourse.tile as tile
from concourse.bass import IndirectOffsetOnAxis

B, L, D = 4, 4096, 1024
DFF = 4 * D
K = L // 2              # 2048 selected tokens per row
NCORES = 8
P = 128
NT = L // P             # 32 token tiles per row
SEL = K // 2            # 1024 selected tokens per core
NSJ = SEL // P          # 8 selected-token blocks
ND = D // P             # 8 d chunks
NM = DFF // P           # 32 dff tiles
NKGRP = 4               # w2 k-chunks per streamed tile
RADIX_PASSES = 4
OOB_SENTINEL = 2 * L    # > bounds_check => skipped by indirect DMA

F32 = mybir.dt.float32
BF16 = mybir.dt.bfloat16
I32 = mybir.dt.int32
Alu = mybir.AluOpType
Act = mybir.ActivationFunctionType


def build_program():
    nc = bacc.Bacc(
        "TRN2",
        target_bir_lowering=False,
        debug=False,
        enable_asserts=False,
        num_devices=NCORES,
    )

    x_row = nc.dram_tensor("x_row", [L, D], F32, kind="ExternalInput").ap()
    w1 = nc.dram_tensor("w1", [D, DFF], F32, kind="ExternalInput").ap()
    w2 = nc.dram_tensor("w2", [DFF, D], F32, kind="ExternalInput").ap()
    wr = nc.dram_tensor("wr", [1, D], F32, kind="ExternalInput").ap()
    br = nc.dram_tensor("br", [1, 1], F32, kind="ExternalInput").ap()
    b1t = nc.dram_tensor("b1t", [P, NM], F32, kind="ExternalInput").ap()
    b2 = nc.dram_tensor("b2", [1, D], F32, kind="ExternalInput").ap()
    hbase = nc.dram_tensor("hbase", [1, 1], F32, kind="ExternalInput").ap()
    ident = nc.dram_tensor("ident128", [P, P], F32, kind="ExternalInput").ap()
    ltri = nc.dram_tensor("ltri128", [P, P], F32, kind="ExternalInput").ap()
    slt32 = nc.dram_tensor("slt32", [NT, NT], F32, kind="ExternalInput").ap()
    id32 = nc.dram_tensor("id32", [NT, NT], F32, kind="ExternalInput").ap()
    ones_1x128 = nc.dram_tensor("ones_1x128", [1, P], F32, kind="ExternalInput").ap()
    ones_1x128b = nc.dram_tensor("ones_1x128b", [1, P], BF16, kind="ExternalInput").ap()
    ones_128x1 = nc.dram_tensor("ones_128x1", [P, 1], F32, kind="ExternalInput").ap()
    ones_32x128 = nc.dram_tensor("ones_32x128", [NT, P], F32, kind="ExternalInput").ap()

    out_row = nc.dram_tensor("out_row", [L, D], F32, kind="ExternalOutput").ap()

    scores_d = nc.dram_tensor("scores_d", [P, NT], F32).ap()
    selidx2_d = nc.dram_tensor("selidx2_d", [SEL, 1], F32).ap()

    with tile.TileContext(nc) as tc, ExitStack() as S0:
        const = S0.enter_context(tc.tile_pool(name="const", bufs=1))
        w1_pool = S0.enter_context(tc.tile_pool(name="w1bf", bufs=1))

        # ---- small constant loads ------------------------------------------------
        def cload(pool, ap, shape, dtype=F32, name=None):
            t = pool.tile(shape, dtype, name=name)
            nc.sync.dma_start(out=t[:], in_=ap)
            return t

        br_sb = cload(const, br, [1, 1], name="c_br")
        hb_sb = cload(const, hbase, [1, 1], name="c_hb")
        b1t_sb = cload(const, b1t, [P, NM], name="c_b1t")
        ident_sb = cload(const, ident, [P, P], name="c_id")
        ltri_sb = cload(const, ltri, [P, P], name="c_lt")
        slt32_sb = cload(const, slt32, [NT, NT], name="c_sl")
        id32_sb = cload(const, id32, [NT, NT], name="c_id32")
        o1x128_sb = cload(const, ones_1x128, [1, P], name="c_o1")
        o1x128b_sb = cload(const, ones_1x128b, [1, P], BF16, name="c_o1b")
        o128x1_sb = cload(const, ones_128x1, [P, 1], name="c_oc")
        o32x128_sb = cload(const, ones_32x128, [NT, P], name="c_o32")
        b2bf_sb = const.tile([1, D], BF16)
        nc.gpsimd.dma_start(out=b2bf_sb[:], in_=b2)  # cast f32 -> bf16

        br_col = const.tile([P, 1], F32)
        nc.gpsimd.partition_broadcast(br_col[:], br_sb[:])
        hb_col = const.tile([P, 1], F32)
        nc.gpsimd.partition_broadcast(hb_col[:], hb_sb[:])

        iota_i = const.tile([P, 1], I32)
        nc.gpsimd.iota(iota_i[:], pattern=[[1, 1]], base=0, channel_multiplier=1)
        iota_f = const.tile([P, 1], F32)
        nc.vector.tensor_copy(out=iota_f[:], in_=iota_i[:])

        tokid = const.tile([P, NT], I32)
        nc.gpsimd.iota(tokid[:], pattern=[[P, NT]], base=0, channel_multiplier=1)
        iota512 = const.tile([P, 512], I32)
        nc.gpsimd.iota(iota512[:], pattern=[[1, 512]], base=0, channel_multiplier=0)
        iota512f = const.tile([P, 512], F32)
        nc.vector.tensor_copy(out=iota512f[:], in_=iota512[:])
        tokidf = const.tile([P, NT], F32)
        nc.vector.tensor_copy(out=tokidf[:], in_=tokid[:])

        scores_sb = const.tile([P, NT], F32)
        selidx_sb = const.tile([P, NSJ], I32)
        offf_c = const.tile([P, NT], F32)

        # ---- w1 resident loads (cast f32->bf16 during DMA), overlap prefix ------
        w1bf = []
        for kd in range(ND):
            t_ = w1_pool.tile([P, DFF], BF16, name=f"w1bf_{kd}")
            nc.gpsimd.dma_start(out=t_[:], in_=w1[kd * P:(kd + 1) * P, :])
            w1bf.append(t_)

        if os.environ.get("KVAR") == "noprefix":
            # diagnostic variant: synthetic selection (first 1024 tokens)
            offi_tmp = const.tile([P, NT], I32)
            nc.vector.memset(offi_tmp[:], OOB_SENTINEL)
            nc.gpsimd.iota(offi_tmp[:, :NSJ], pattern=[[P, NSJ]], base=0,
                           channel_multiplier=1)
            nc.vector.tensor_copy(out=offf_c[:], in_=offi_tmp[:])
        if os.environ.get("KVAR") != "noprefix":
          with ExitStack() as SPM:
            misc_psum = SPM.enter_context(tc.tile_pool(name="misc_psum", bufs=2, space="PSUM"))

            # ---- phase A: router scores (fp32, exact) ----------------------------
            with ExitStack() as SA:
                apool = SA.enter_context(tc.tile_pool(name="apool", bufs=1))
                xs_pool = SA.enter_context(tc.tile_pool(name="xs", bufs=5))
                junk_pool = SA.enter_context(tc.tile_pool(name="junk", bufs=2))

                wr_sb = cload(apool, wr, [1, D], name="c_wr")
                wrb = apool.tile([P, D], F32)
                for n in range(D // 512):
                    pt = misc_psum.tile([P, 512], F32, name="mp")
                    nc.tensor.matmul(out=pt[:], lhsT=o1x128_sb[:],
                                     rhs=wr_sb[:, n * 512:(n + 1) * 512],
                                     start=True, stop=True)
                    nc.vector.tensor_copy(out=wrb[:, n * 512:(n + 1) * 512], in_=pt[:])

                for t in range(NT):
                    x_t = xs_pool.tile([P, D], F32)
                    nc.sync.dma_start(out=x_t[:], in_=x_row[t * P:(t + 1) * P, :])
                    prod = junk_pool.tile([P, D], F32, name="prod")
                    nc.vector.tensor_tensor(out=prod[:], in0=x_t[:], in1=wrb[:],
                                            op=Alu.mult)
                    sink = junk_pool.tile([P, D], BF16, name="sink")
                    nc.scalar.activation(out=sink[:], in_=prod[:], func=Act.Identity,
                                         bias=0.0, scale=1.0,
                                         accum_out=scores_sb[:, t:t + 1])
                # add router bias once
                scores2 = apool.tile([P, NT], F32, name="scores2")
                nc.vector.tensor_tensor(out=scores2[:], in0=scores_sb[:],
                                        in1=br_col[:, :1].to_broadcast([P, NT]),
                                        op=Alu.add)
                nc.vector.tensor_copy(out=scores_sb[:], in_=scores2[:])

            # ---- phase C: top-K threshold via 128-way bisection ------------------
            with ExitStack() as SC:
                radix = SC.enter_context(tc.tile_pool(name="radix", bufs=2))
                rjunk = SC.enter_context(tc.tile_pool(name="rjunk", bufs=1))
                rep_pool = SC.enter_context(tc.tile_pool(name="rep", bufs=1))

                nc.sync.dma_start(out=scores_d, in_=scores_sb[:])
                scores_row = rep_pool.tile([1, L], F32)
                nc.sync.dma_start(out=scores_row[:], in_=scores_d.rearrange("p c -> () (p c)"))
                scores_rep = rep_pool.tile([P, L], F32)
                for n in range(L // 512):
                    pt = misc_psum.tile([P, 512], F32, name="mp")
                    nc.tensor.matmul(out=pt[:], lhsT=o1x128_sb[:],
                                     rhs=scores_row[:, n * 512:(n + 1) * 512],
                                     start=True, stop=True)
                    nc.vector.tensor_copy(out=scores_rep[:, n * 512:(n + 1) * 512], in_=pt[:])

                lo = radix.tile([1, 1], F32, name="lo")
                nc.vector.memset(lo[:], -16.0)
                w_ = radix.tile([1, 1], F32, name="w")
                nc.vector.memset(w_[:], 32.0 / P)
                thrb = radix.tile([P, 1], F32, name="thrb")
                nc.vector.tensor_scalar(out=thrb[:], in0=iota_f[:], scalar1=32.0 / P,
                                        scalar2=None, op0=Alu.mult)
                for _pass in range(RADIX_PASSES):
                    lo_c = radix.tile([P, 1], F32, name="lo_c")
                    nc.gpsimd.partition_broadcast(lo_c[:], lo[:])
                    thr2 = radix.tile([P, 1], F32, name="thr2")
                    nc.vector.tensor_tensor(out=thr2[:], in0=thrb[:], in1=lo_c[:], op=Alu.add)
                    cnt = radix.tile([P, 1], F32, name="cnt")
                    junk2 = rjunk.tile([P, L], F32, name="junk2")
                    nc.vector.tensor_tensor(out=junk2[:], in0=scores_rep[:],
                                            in1=thr2[:, :1].to_broadcast([P, L]),
                                            op=Alu.is_ge)
                    sink2 = rjunk.tile([P, L], BF16, name="sink2")
                    nc.scalar.activation(out=sink2[:], in_=junk2[:], func=Act.Identity,
                                         bias=0.0, scale=1.0, accum_out=cnt[:])
                    sel = radix.tile([P, 1], F32, name="sel")
                    nc.vector.tensor_scalar(out=sel[:], in0=cnt[:], scalar1=float(K),
                                            scalar2=None, op0=Alu.is_ge)
                    ssum_p = misc_psum.tile([1, 1], F32, name="mp")
                    nc.tensor.matmul(out=ssum_p[:], lhsT=sel[:], rhs=o128x1_sb[:],
                                     start=True, stop=True)
                    s_sb = radix.tile([1, 1], F32, name="s_sb")
                    nc.vector.tensor_copy(out=s_sb[:], in_=ssum_p[:])
                    ps = radix.tile([1, 1], F32, name="ps")
                    nc.vector.tensor_scalar(out=ps[:], in0=s_sb[:], scalar1=-1.0,
                                            scalar2=None, op0=Alu.add)
                    d_ = radix.tile([1, 1], F32, name="d_")
                    nc.vector.tensor_tensor(out=d_[:], in0=ps[:], in1=w_[:], op=Alu.mult)
                    lo2 = radix.tile([1, 1], F32, name="lo2")
                    nc.vector.tensor_tensor(out=lo2[:], in0=lo[:], in1=d_[:], op=Alu.add)
                    w2_ = radix.tile([1, 1], F32, name="w2_")
                    nc.vector.tensor_scalar(out=w2_[:], in0=w_[:], scalar1=1.0 / P,
                                            scalar2=None, op0=Alu.mult)
                    thrb2 = radix.tile([P, 1], F32, name="thrb")
                    nc.vector.tensor_scalar(out=thrb2[:], in0=thrb[:], scalar1=1.0 / P,
                                            scalar2=None, op0=Alu.mult)
                    lo, w_, thrb = lo2, w2_, thrb2

                # ---- mask, global rank, rank-window compaction -------------------
                T_col = radix.tile([P, 1], F32, name="T_col")
                nc.gpsimd.partition_broadcast(T_col[:], lo[:])
                maskf = radix.tile([P, NT], F32, name="maskf")
                nc.vector.tensor_tensor(out=maskf[:], in0=scores_sb[:],
                                        in1=T_col[:, :1].to_broadcast([P, NT]), op=Alu.is_ge)

                colsum_p = misc_psum.tile([NT, 1], F32, name="mp")
                nc.tensor.matmul(out=colsum_p[:], lhsT=maskf[:], rhs=o128x1_sb[:],
                                 start=True, stop=True)
                colsum = radix.tile([NT, 1], F32, name="colsum")
                nc.vector.tensor_copy(out=colsum[:], in_=colsum_p[:])
                excl_p = misc_psum.tile([NT, 1], F32, name="mp")
                nc.tensor.matmul(out=excl_p[:], lhsT=slt32_sb[:], rhs=colsum[:],
                                 start=True, stop=True)
                excl = radix.tile([NT, 1], F32, name="excl")
                nc.vector.tensor_copy(out=excl[:], in_=excl_p[:])
                diag = radix.tile([NT, NT], F32, name="diag")
                nc.vector.tensor_tensor(out=diag[:], in0=id32_sb[:],
                                        in1=excl[:, :1].to_broadcast([NT, NT]), op=Alu.mult)
                rank_p = misc_psum.tile([P, NT], F32, name="mp")
                nc.tensor.matmul(out=rank_p[:], lhsT=ltri_sb[:], rhs=maskf[:],
                                 start=True, stop=False, skip_group_check=True)
                nc.tensor.matmul(out=rank_p[:], lhsT=o32x128_sb[:], rhs=diag[:],
                                 start=False, stop=True, skip_group_check=True)
                rank = radix.tile([P, NT], F32, name="rank")
                nc.vector.tensor_copy(out=rank[:], in_=rank_p[:])

                off = radix.tile([P, NT], F32, name="off")
                nc.vector.tensor_tensor(out=off[:], in0=rank[:],
                                        in1=hb_col[:, :1].to_broadcast([P, NT]),
                                        op=Alu.subtract)
                w0 = radix.tile([P, NT], F32, name="w0")
                nc.vector.tensor_scalar(out=w0[:], in0=off[:], scalar1=0.0, scalar2=None,
                                        op0=Alu.is_ge)
                w1m = radix.tile([P, NT], F32, name="w1m")
                nc.vector.tensor_scalar(out=w1m[:], in0=off[:], scalar1=float(SEL),
                                        scalar2=None, op0=Alu.is_lt)
                m2 = radix.tile([P, NT], F32, name="m2")
                nc.vector.tensor_tensor(out=m2[:], in0=w0[:], in1=w1m[:], op=Alu.mult)
                m3 = radix.tile([P, NT], F32, name="m3")
                nc.vector.tensor_tensor(out=m3[:], in0=m2[:], in1=maskf[:], op=Alu.mult)
                t1 = radix.tile([P, NT], F32, name="t1")
                nc.vector.tensor_scalar(out=t1[:], in0=off[:],
                                        scalar1=-float(OOB_SENTINEL),
                                        scalar2=None, op0=Alu.add)
                t2 = radix.tile([P, NT], F32, name="t2")
                nc.vector.tensor_tensor(out=t2[:], in0=t1[:], in1=m3[:], op=Alu.mult)
                offf = radix.tile([P, NT], F32, name="offf")
                nc.vector.tensor_scalar(out=offf[:], in0=t2[:],
                                        scalar1=float(OOB_SENTINEL),
                                        scalar2=None, op0=Alu.add)
                nc.vector.tensor_copy(out=offf_c[:], in_=offf[:])

        # ---- compaction: sel_idx[r] = token id with rank r, via selection matmul -
        # S[p, r] = (offf[p, c] == r) is one-hot per rank; tokid_col^T @ S
        # accumulated over the 32 token chunks yields the compacted index row.
        # Output DRAM buffers arrive pre-zeroed (run_bass_via_pjrt donates
        # np.zeros buffers; native run_neff pre-zeros out_maps), so unselected
        # out_row rows stay zero without an explicit fill.
        with ExitStack() as SG:
            sg_pool = SG.enter_context(tc.tile_pool(name="sg", bufs=3))
            sg_psum = SG.enter_context(tc.tile_pool(name="sg_psum", bufs=2, space="PSUM"))
            sel_ps = [sg_psum.tile([1, 512], F32, name="selps") for _ in range(2)]
            for c in range(NT):
                offc = sg_pool.tile([P, 1], F32, name="offc")
                nc.vector.tensor_copy(out=offc[:], in_=offf_c[:, c:c + 1])
                for n in range(2):
                    on = sg_pool.tile([P, 1], F32, name="on")
                    nc.vector.tensor_scalar(out=on[:], in0=offc[:],
                                            scalar1=-float(n * 512), scalar2=None,
                                            op0=Alu.add)
                    smat = sg_pool.tile([P, 512], F32, name="smat")
                    nc.vector.tensor_tensor(out=smat[:], in0=iota512f[:],
                                            in1=on[:, :1].to_broadcast([P, 512]),
                                            op=Alu.is_equal)
                    nc.tensor.matmul(out=sel_ps[n][:], lhsT=tokidf[:, c:c + 1],
                                     rhs=smat[:], start=(c == 0), stop=(c == NT - 1),
                                     skip_group_check=True)
            selrow = sg_pool.tile([1, SEL], F32, name="selrow")
            for n in range(2):
                nc.vector.tensor_copy(out=selrow[:, n * 512:(n + 1) * 512],
                                      in_=sel_ps[n][:])
            nc.sync.dma_start(out=selidx2_d, in_=selrow[:])
            # reload as [P, NSJ] with (p, j) <- rank j*128 + p, cast to int32
            self_sb = sg_pool.tile([P, NSJ], F32, name="self_sb")
            nc.sync.dma_start(
                out=self_sb[:],
                in_=selidx2_d.rearrange("(j p) one -> p (j one)", p=P))
            nc.vector.tensor_copy(out=selidx_sb[:], in_=self_sb[:])

        # ---- gather + transpose + MLP --------------------------------------------
        with ExitStack() as SM:
            ht_pool = SM.enter_context(tc.tile_pool(name="ht", bufs=1))
            ht = ht_pool.tile([P, NM, SEL], BF16)

            with ExitStack() as SB:
                xt_pool = SB.enter_context(tc.tile_pool(name="xt", bufs=1))
                xsel_pool = SB.enter_context(tc.tile_pool(name="xsel", bufs=4))
                tp_psum = SB.enter_context(tc.tile_pool(name="tp_psum", bufs=2, space="PSUM"))
                mm1_psum = SB.enter_context(tc.tile_pool(name="mm1_psum", bufs=6, space="PSUM"))

                xt = []
                for kd in range(ND):
                    xt.append(xt_pool.tile([P, SEL], BF16, name=f"xt_{kd}"))
                for j in range(NSJ):
                    xs = xsel_pool.tile([P, D], F32, name="xsel")
                    nc.gpsimd.indirect_dma_start(
                        out=xs[:], out_offset=None, in_=x_row,
                        in_offset=IndirectOffsetOnAxis(ap=selidx_sb[:, j:j + 1],
                                                       axis=0))
                    for kd in range(ND):
                        tp = tp_psum.tile([P, P], F32, name="tp")
                        nc.tensor.transpose(out=tp[:], in_=xs[:, kd * P:(kd + 1) * P],
                                            identity=ident_sb[:])
                        nc.vector.tensor_copy(out=xt[kd][:, j * P:(j + 1) * P], in_=tp[:])

                # ---- mm1: ht[m, sel] = gelu(w1^T x_sel^T + b1) -------------------
                for n in range(SEL // 512):
                    for m in range(NM):
                        ph = mm1_psum.tile([P, 512], F32, name="ph")
                        for kd in range(ND):
                            nc.tensor.matmul(
                                out=ph[:],
                                lhsT=w1bf[kd][:, m * P:(m + 1) * P],
                                rhs=xt[kd][:, n * 512:(n + 1) * 512],
                                start=(kd == 0), stop=(kd == ND - 1),
                            )
                        nc.scalar.activation(
                            out=ht[:, m, n * 512:(n + 1) * 512], in_=ph[:],
                            func=Act.Gelu_apprx_tanh, bias=b1t_sb[:, m:m + 1], scale=1.0,
                        )

            # ---- mm2: y[sel, D] = ht^T @ w2 + b2 ---------------------------------
            if os.environ.get("KVAR") == "mm1stop":
                SM.close()
                nc.compile()
                return nc
            with ExitStack() as SY:
                y_pool = SY.enter_context(tc.tile_pool(name="y", bufs=1))
                w2_pool = SY.enter_context(tc.tile_pool(name="w2s", bufs=5))
                mm2_psum = SY.enter_context(tc.tile_pool(name="mm2_psum", bufs=8, space="PSUM"))
                y_sb = y_pool.tile([P, NSJ, D], F32)
                for n in range(D // 512):
                    pys = [mm2_psum.tile([P, 512], F32, name="py") for _ in range(NSJ)]
                    for s in range(NSJ):
                        nc.tensor.matmul(
                            out=pys[s][:], lhsT=o1x128b_sb[:],
                            rhs=b2bf_sb[:, n * 512:(n + 1) * 512],
                            start=True, stop=False, skip_group_check=True,
                        )
                    for kg in range(NM // NKGRP):
                        w2t = w2_pool.tile([P, NKGRP, 512], BF16, name="w2t")
                        src = w2[:, n * 512:(n + 1) * 512].rearrange(
                            "(g p) f -> p g f", p=P)[:, kg * NKGRP:(kg + 1) * NKGRP, :]
                        nc.gpsimd.dma_start(out=w2t[:], in_=src)
                        for ki in range(NKGRP):
                            kk = kg * NKGRP + ki
                            for s in range(NSJ):
                                nc.tensor.matmul(
                                    out=pys[s][:],
                                    lhsT=ht[:, kk, s * P:(s + 1) * P],
                                    rhs=w2t[:, ki, :],
                                    start=False, stop=(kk == NM - 1),
                                    skip_group_check=True,
                                )
                    for s in range(NSJ):
                        nc.vector.tensor_copy(
                            out=y_sb[:, s, n * 512:(n + 1) * 512], in_=pys[s][:])

                # ---- scatter y rows into zeroed output ---------------------------
                for j in range(NSJ):
                    nc.gpsimd.indirect_dma_start(
                        out=out_row, out_offset=IndirectOffsetOnAxis(
                            ap=selidx_sb[:, j:j + 1], axis=0),
                        in_=y_sb[:, j, :], in_offset=None,
                    )

    nc.compile()
    return nc


def make_consts():
    q = np.arange(P)
    consts = {
        "ident128": np.eye(P, dtype=np.float32),
        "ltri128": (q[:, None] < q[None, :]).astype(np.float32),  # [q, p] = q < p
        "slt32": (np.arange(NT)[:, None] < np.arange(NT)[None, :]).astype(np.float32),
        "id32": np.eye(NT, dtype=np.float32),
        "ones_1x128": np.ones((1, P), np.float32),
        "ones_128x1": np.ones((P, 1), np.float32),
        "ones_32x128": np.ones((NT, P), np.float32),
    }
    import ml_dtypes
    consts["ones_1x128b"] = np.ones((1, P), ml_dtypes.bfloat16)
    return consts


def make_in_maps(x, W1, b1, W2, b2, wr, br):
    consts = make_consts()
    x = np.ascontiguousarray(np.asarray(x, np.float32))
    in_maps = []
    for c in range(NCORES):
        b, h = divmod(c, 2)
        m = {
            "x_row": x[b],
            "w1": np.asarray(W1, np.float32),
            "w2": np.asarray(W2, np.float32),
            "wr": np.asarray(wr, np.float32).reshape(1, D),
            "br": np.asarray(br, np.float32).reshape(1, 1),
            "b1t": np.ascontiguousarray(np.asarray(b1, np.float32).reshape(NM, P).T),
            "b2": np.asarray(b2, np.float32).reshape(1, D),
            "hbase": np.array([[h * SEL]], np.float32),
        }
        m.update(consts)
        in_maps.append(m)
    return in_maps


_NC_CACHE = None


def _get_program():
    global _NC_CACHE
    if _NC_CACHE is None:
        _NC_CACHE = build_program()
    return _NC_CACHE


def kernel(x, W1, b1, W2, b2, wr, br):
    from concourse.bass_utils import run_bass_kernel_spmd

    nc = _get_program()
    in_maps = make_in_maps(x, W1, b1, W2, b2, wr, br)
    res = run_bass_kernel_spmd(nc, in_maps, list(range(NCORES))).results
    out = np.stack(
        [res[2 * b]["out_row"] + res[2 * b + 1]["out_row"] for b in range(B)]
    )
    return out.astype(np.float32)



# revision 17
# speedup vs baseline: 2.3775x; 2.3775x over previous
"""MoD (mixture-of-depths) MLP wrapper kernel for Trainium2, 8 NeuronCores.

Sharding: core c handles batch row b = c//2 and the half of that row's
top-K tokens with global selection ranks in [h*1024, (h+1)*1024), h = c%2.
Each core computes the full row's router scores + top-K threshold locally
(no collectives), inverts rank->token via ONE indirect-scatter DMA, gathers
its 1024 token rows (bf16 cast in DMA), runs the FFN in bf16 (fp32
accumulation), and writes a compact [1024, D] result + the token ids.
The host places rows at their token positions while unsharding.
"""

import sys

sys.path.insert(0, "/opt/trn_rl_repo")

from contextlib import ExitStack

import numpy as np

from concourse import bass, mybir
from concourse import bacc
import concourse.tile as tile
from concourse.bass import IndirectOffsetOnAxis

B, L, D = 4, 4096, 1024
DFF = 4 * D
K = L // 2              # 2048 selected tokens per row
NCORES = 8
P = 128
NT = L // P             # 32 token tiles per row
SEL = K // 2            # 1024 selected tokens per core
NSJ = SEL // P          # 8 selected-token blocks
ND = D // P             # 8 d chunks
NM = DFF // P           # 32 dff tiles
NKGRP = 4               # w2 k-chunks per streamed tile
RADIX_PASSES = 4
OOB_SENTINEL = 2 * L    # > bounds_check => skipped by indirect DMA

F32 = mybir.dt.float32
BF16 = mybir.dt.bfloat16
FP16 = mybir.dt.float16
I32 = mybir.dt.int32
Alu = mybir.AluOpType
Act = mybir.ActivationFunctionType


def build_program():
    nc = bacc.Bacc(
        "TRN2",
        target_bir_lowering=False,
        debug=False,
        enable_asserts=False,
        num_devices=NCORES,
    )

    x_row = nc.dram_tensor("x_row", [L, D], F32, kind="ExternalInput").ap()
    w1b = nc.dram_tensor("w1b", [D, DFF], BF16, kind="ExternalInput").ap()
    w2b = nc.dram_tensor("w2b", [DFF, D], BF16, kind="ExternalInput").ap()
    wr = nc.dram_tensor("wr", [1, D], F32, kind="ExternalInput").ap()
    b1t = nc.dram_tensor("b1t", [P, NM], F32, kind="ExternalInput").ap()
    b2b = nc.dram_tensor("b2b", [1, D], BF16, kind="ExternalInput").ap()
    hbase = nc.dram_tensor("hbase", [1, 1], F32, kind="ExternalInput").ap()
    ident = nc.dram_tensor("ident128", [P, P], F32, kind="ExternalInput").ap()
    identb = nc.dram_tensor("identb128", [P, P], BF16, kind="ExternalInput").ap()
    ltri = nc.dram_tensor("ltri128", [P, P], F32, kind="ExternalInput").ap()
    slt32 = nc.dram_tensor("slt32", [NT, NT], F32, kind="ExternalInput").ap()
    id32 = nc.dram_tensor("id32", [NT, NT], F32, kind="ExternalInput").ap()
    ones_1x128 = nc.dram_tensor("ones_1x128", [1, P], F32, kind="ExternalInput").ap()
    ones_1x128b = nc.dram_tensor("ones_1x128b", [1, P], BF16, kind="ExternalInput").ap()
    ones_128x1 = nc.dram_tensor("ones_128x1", [P, 1], F32, kind="ExternalInput").ap()
    ones_32x128 = nc.dram_tensor("ones_32x128", [NT, P], F32, kind="ExternalInput").ap()
    abh = nc.dram_tensor("abh", [P, 64 * NT], FP16, kind="ExternalInput").ap()

    y_d = nc.dram_tensor("y_d", [SEL, D], BF16, kind="ExternalOutput").ap()
    sel_d = nc.dram_tensor("sel_d", [SEL, 1], F32, kind="ExternalOutput").ap()

    with tile.TileContext(nc) as tc, ExitStack() as S0:
        const = S0.enter_context(tc.tile_pool(name="const", bufs=1))
        w1_pool = S0.enter_context(tc.tile_pool(name="w1bf", bufs=1))

        # ---- small constant loads (sync queue, ahead of x tiles) ---------------
        def cload(pool, ap, shape, dtype=F32, name=None):
            t = pool.tile(shape, dtype, name=name)
            nc.sync.dma_start(out=t[:], in_=ap)
            return t

        wr_sb = cload(const, wr, [1, D], name="c_wr")
        b1t_sb = cload(const, b1t, [P, NM], name="c_b1t")
        b2b_sb = cload(const, b2b, [1, D], BF16, name="c_b2b")
        hb_sb = cload(const, hbase, [1, 1], name="c_hb")
        ident_sb = cload(const, ident, [P, P], name="c_id")
        identb_sb = cload(const, identb, [P, P], BF16, name="c_idb")
        ltri_sb = cload(const, ltri, [P, P], name="c_lt")
        slt32_sb = cload(const, slt32, [NT, NT], name="c_sl")
        id32_sb = cload(const, id32, [NT, NT], name="c_id32")
        o1x128_sb = cload(const, ones_1x128, [1, P], name="c_o1")
        o1x128b_sb = cload(const, ones_1x128b, [1, P], BF16, name="c_o1b")
        o128x1_sb = cload(const, ones_128x1, [P, 1], name="c_oc")
        o32x128_sb = cload(const, ones_32x128, [NT, P], name="c_o32")
        abh_sb = cload(const, abh, [P, 64 * NT], FP16, name="c_abh")

        iota1024_i = const.tile([P, SEL], I32)
        nc.gpsimd.iota(iota1024_i[:], pattern=[[1, SEL]], base=0,
                       channel_multiplier=0)
        iota1024h = const.tile([P, SEL], FP16)
        nc.vector.tensor_copy(out=iota1024h[:], in_=iota1024_i[:])

        hb_col = const.tile([P, 1], F32)
        nc.gpsimd.partition_broadcast(hb_col[:], hb_sb[:])

        iota_i = const.tile([P, 1], I32)
        nc.gpsimd.iota(iota_i[:], pattern=[[1, 1]], base=0, channel_multiplier=1)
        iota_f = const.tile([P, 1], F32)
        nc.vector.tensor_copy(out=iota_f[:], in_=iota_i[:])

        tokid = const.tile([P, NT], I32)
        nc.gpsimd.iota(tokid[:], pattern=[[P, NT]], base=0, channel_multiplier=1)
        tokidf = const.tile([P, NT], F32)
        nc.vector.tensor_copy(out=tokidf[:], in_=tokid[:])

        scores_sb = const.tile([P, NT], F32)
        selidx_sb = const.tile([P, NSJ], I32)

        # ---- phase A: router scores (fp32, exact) + replicated score matrix ----
        with ExitStack() as SREP:
            rep_pool = SREP.enter_context(tc.tile_pool(name="rep", bufs=1))
            scores_row = rep_pool.tile([1, L], F32)
            scores_rep = rep_pool.tile([P, L], F32)

            with ExitStack() as SA:
                apool = SA.enter_context(tc.tile_pool(name="apool", bufs=1))
                xs_pool = SA.enter_context(tc.tile_pool(name="xs", bufs=5))
                junk_pool = SA.enter_context(tc.tile_pool(name="junk", bufs=2))
                pa_psum = SA.enter_context(tc.tile_pool(name="pa_psum", bufs=2, space="PSUM"))

                wrb = apool.tile([P, D], F32)
                for n in range(D // 512):
                    pt = pa_psum.tile([P, 512], F32, name="pa_mp")
                    nc.tensor.matmul(out=pt[:], lhsT=o1x128_sb[:],
                                     rhs=wr_sb[:, n * 512:(n + 1) * 512],
                                     start=True, stop=True)
                    nc.vector.tensor_copy(out=wrb[:, n * 512:(n + 1) * 512], in_=pt[:])

                for t in range(NT):
                    x_t = xs_pool.tile([P, D], F32)
                    nc.sync.dma_start(out=x_t[:], in_=x_row[t * P:(t + 1) * P, :])
                    prod = junk_pool.tile([P, D], F32, name="prod")
                    nc.vector.tensor_tensor(out=prod[:], in0=x_t[:], in1=wrb[:],
                                            op=Alu.mult)
                    sink = junk_pool.tile([P, D], BF16, name="sink")
                    nc.scalar.activation(out=sink[:], in_=prod[:], func=Act.Identity,
                                         bias=0.0, scale=1.0,
                                         accum_out=scores_sb[:, t:t + 1])
                    # transpose the fresh score column into the [1, L] row
                    tpp = pa_psum.tile([1, P], F32, name="pa_tp")
                    nc.tensor.transpose(out=tpp[:], in_=scores_sb[:, t:t + 1],
                                        identity=ident_sb[:])
                    nc.vector.tensor_copy(out=scores_row[:, t * P:(t + 1) * P],
                                          in_=tpp[:])
                    if t % 4 == 3:
                        n = t // 4
                        bp = pa_psum.tile([P, 512], F32, name="pa_mp")
                        nc.tensor.matmul(out=bp[:], lhsT=o1x128_sb[:],
                                         rhs=scores_row[:, n * 512:(n + 1) * 512],
                                         start=True, stop=True)
                        nc.vector.tensor_copy(
                            out=scores_rep[:, n * 512:(n + 1) * 512], in_=bp[:])

            # ---- w1 resident loads (bf16, queued behind x on sync) --------------
            w1bf = []
            for kd in range(ND):
                t_ = w1_pool.tile([P, DFF], BF16, name=f"w1bf_{kd}")
                nc.sync.dma_start(out=t_[:], in_=w1b[kd * P:(kd + 1) * P, :])
                w1bf.append(t_)

            # ---- phase C: top-K threshold via 128-way bisection (Sign counts) ---
            with ExitStack() as SC:
                radix = SC.enter_context(tc.tile_pool(name="radix", bufs=2))
                rjunk = SC.enter_context(tc.tile_pool(name="rjunk", bufs=2))
                rx_psum = SC.enter_context(tc.tile_pool(name="rx_psum", bufs=1, space="PSUM"))

                neglo = radix.tile([P, 1], F32, name="neglo")
                nc.vector.memset(neglo[:], 16.0)
                w_cur = 32.0 / P
                for _pass in range(RADIX_PASSES):
                    negthr = radix.tile([P, 1], F32, name="negthr")
                    nc.vector.tensor_scalar(out=negthr[:], in0=iota_f[:],
                                            scalar1=-w_cur, scalar2=neglo[:],
                                            op0=Alu.mult, op1=Alu.add)
                    sumsign = radix.tile([P, 1], F32, name="sumsign")
                    sink2 = rjunk.tile([P, L], BF16, name="sink2")
                    nc.scalar.activation(out=sink2[:], in_=scores_rep[:],
                                         func=Act.Sign, bias=negthr[:], scale=1.0,
                                         accum_out=sumsign[:])
                    sel = radix.tile([P, 1], F32, name="sel")
                    nc.vector.tensor_scalar(out=sel[:], in0=sumsign[:], scalar1=0.0,
                                            scalar2=None, op0=Alu.is_ge)
                    s_ps = rx_psum.tile([1, 1], F32, name="s_ps")
                    nc.tensor.matmul(out=s_ps[:], lhsT=sel[:], rhs=o128x1_sb[:],
                                     start=True, stop=True)
                    s_sb = radix.tile([1, 1], F32, name="s_sb")
                    nc.vector.tensor_copy(out=s_sb[:], in_=s_ps[:])
                    bc_ps = rx_psum.tile([P, 1], F32, name="bc_ps")
                    nc.tensor.matmul(out=bc_ps[:], lhsT=o1x128_sb[:], rhs=s_sb[:],
                                     start=True, stop=True)
                    # neglo' = neglo + (1 - s) * w
                    delta = radix.tile([P, 1], F32, name="delta")
                    nc.vector.tensor_scalar(out=delta[:], in0=bc_ps[:],
                                            scalar1=-w_cur, scalar2=w_cur,
                                            op0=Alu.mult, op1=Alu.add)
                    neglo2 = radix.tile([P, 1], F32, name="neglo")
                    nc.vector.tensor_tensor(out=neglo2[:], in0=neglo[:],
                                            in1=delta[:], op=Alu.add)
                    neglo = neglo2
                    w_cur /= P

                T_col = radix.tile([P, 1], F32, name="T_col")
                nc.vector.tensor_scalar(out=T_col[:], in0=neglo[:], scalar1=-1.0,
                                        scalar2=None, op0=Alu.mult)

                # ---- mask, global rank, local scatter offsets --------------------
                maskf = radix.tile([P, NT], F32, name="maskf")
                nc.vector.tensor_scalar(out=maskf[:], in0=scores_sb[:],
                                        scalar1=T_col[:], scalar2=None,
                                        op0=Alu.is_ge)
                colsum_p = rx_psum.tile([NT, 1], F32, name="cs_ps")
                nc.tensor.matmul(out=colsum_p[:], lhsT=maskf[:], rhs=o128x1_sb[:],
                                 start=True, stop=True)
                colsum = radix.tile([NT, 1], F32, name="colsum")
                nc.vector.tensor_copy(out=colsum[:], in_=colsum_p[:])
                excl_p = rx_psum.tile([NT, 1], F32, name="ex_ps")
                nc.tensor.matmul(out=excl_p[:], lhsT=slt32_sb[:], rhs=colsum[:],
                                 start=True, stop=True)
                excl = radix.tile([NT, 1], F32, name="excl")
                nc.vector.tensor_copy(out=excl[:], in_=excl_p[:])
                diag = radix.tile([NT, NT], F32, name="diag")
                nc.vector.tensor_tensor(out=diag[:], in0=id32_sb[:],
                                        in1=excl[:, :1].to_broadcast([NT, NT]),
                                        op=Alu.mult)
                rank_p = rx_psum.tile([P, NT], F32, name="rank_ps")
                nc.tensor.matmul(out=rank_p[:], lhsT=ltri_sb[:], rhs=maskf[:],
                                 start=True, stop=False, skip_group_check=True)
                nc.tensor.matmul(out=rank_p[:], lhsT=o32x128_sb[:], rhs=diag[:],
                                 start=False, stop=True, skip_group_check=True)

                off = radix.tile([P, NT], F32, name="off")
                nc.vector.tensor_scalar(out=off[:], in0=rank_p[:],
                                        scalar1=hb_col[:], scalar2=None,
                                        op0=Alu.subtract)
                w0 = radix.tile([P, NT], F32, name="w0")
                nc.vector.tensor_scalar(out=w0[:], in0=off[:], scalar1=0.0,
                                        scalar2=None, op0=Alu.is_ge)
                w1m = radix.tile([P, NT], F32, name="w1m")
                nc.vector.tensor_scalar(out=w1m[:], in0=off[:], scalar1=float(SEL),
                                        scalar2=None, op0=Alu.is_lt)
                m2 = radix.tile([P, NT], F32, name="m2")
                nc.vector.tensor_tensor(out=m2[:], in0=w0[:], in1=w1m[:], op=Alu.mult)
                m3 = radix.tile([P, NT], F32, name="m3")
                nc.vector.tensor_tensor(out=m3[:], in0=m2[:], in1=maskf[:], op=Alu.mult)
                t1 = radix.tile([P, NT], F32, name="t1")
                nc.vector.tensor_scalar(out=t1[:], in0=off[:],
                                        scalar1=-float(OOB_SENTINEL),
                                        scalar2=None, op0=Alu.add)
                t2 = radix.tile([P, NT], F32, name="t2")
                nc.vector.tensor_tensor(out=t2[:], in0=t1[:], in1=m3[:], op=Alu.mult)
                offf = radix.tile([P, NT], F32, name="offf")
                nc.vector.tensor_scalar(out=offf[:], in0=t2[:],
                                        scalar1=float(OOB_SENTINEL),
                                        scalar2=None, op0=Alu.add)
                # ---- rank -> token-id inversion: fp16 one-hot compaction ---------
                # smat_c[p, r] = (local_rank[p, c] == r) is one-hot per rank r.
                # lhsT col 0 = iota_p, col 32 = 128*c (both exact in fp16), so
                # psum row 0 = A[r] = p*, row 32 = B[r] = 128*c*; token id
                # = A + B. Engine partition starts must be multiples of 32.
                sel_ps = [rx_psum.tile([33, 512], F32, name=f"selps{n2}")
                          for n2 in range(2)]
                for c in range(NT):
                    smat = rjunk.tile([P, SEL], FP16, name="smat")
                    nc.vector.tensor_scalar(out=smat[:], in0=iota1024h[:],
                                            scalar1=offf[:, c:c + 1], scalar2=None,
                                            op0=Alu.is_equal)
                    for n2 in range(2):
                        nc.tensor.matmul(
                            out=sel_ps[n2][:], lhsT=abh_sb[:, 64 * c:64 * c + 33],
                            rhs=smat[:, n2 * 512:(n2 + 1) * 512],
                            start=(c == 0), stop=(c == NT - 1),
                            skip_group_check=True)
                a_sb = radix.tile([1, SEL], F32, name="a_sb")
                selrow = radix.tile([1, SEL], F32, name="selrow")
                for n2 in range(2):
                    nc.vector.tensor_copy(out=a_sb[:, n2 * 512:(n2 + 1) * 512],
                                          in_=sel_ps[n2][0:1, :])
                for n2 in range(2):
                    nc.vector.tensor_tensor(out=selrow[:, n2 * 512:(n2 + 1) * 512],
                                            in0=a_sb[:, n2 * 512:(n2 + 1) * 512],
                                            in1=sel_ps[n2][32:33, :], op=Alu.add)
                nc.sync.dma_start(out=sel_d, in_=selrow[:])
                # reload as [P, NSJ] with (p, j) <- rank j*128 + p, cast to int32
                self_sb = radix.tile([P, NSJ], F32, name="self_sb")
                nc.sync.dma_start(
                    out=self_sb[:],
                    in_=sel_d.rearrange("(j p) one -> p (j one)", p=P))
                nc.vector.tensor_copy(out=selidx_sb[:], in_=self_sb[:])

        # ---- gather (bf16 cast in DMA) + transpose + MLP -----------------------
        with ExitStack() as SM:
            ht_pool = SM.enter_context(tc.tile_pool(name="ht", bufs=1))
            ht = ht_pool.tile([P, NM, SEL], BF16)

            with ExitStack() as SB:
                xt_pool = SB.enter_context(tc.tile_pool(name="xt", bufs=1))
                xsel_pool = SB.enter_context(tc.tile_pool(name="xsel", bufs=4))
                tp_psum = SB.enter_context(tc.tile_pool(name="tp_psum", bufs=2, space="PSUM"))
                mm1_psum = SB.enter_context(tc.tile_pool(name="mm1_psum", bufs=6, space="PSUM"))

                xt = []
                for kd in range(ND):
                    xt.append(xt_pool.tile([P, SEL], BF16, name=f"xt_{kd}"))
                for j in range(NSJ):
                    xs = xsel_pool.tile([P, D], BF16, name="xsel")
                    nc.gpsimd.indirect_dma_start(
                        out=xs[:], out_offset=None, in_=x_row,
                        in_offset=IndirectOffsetOnAxis(ap=selidx_sb[:, j:j + 1],
                                                       axis=0))
                    for kd in range(ND):
                        tp = tp_psum.tile([P, P], BF16, name="tp")
                        nc.tensor.transpose(out=tp[:], in_=xs[:, kd * P:(kd + 1) * P],
                                            identity=identb_sb[:])
                        nc.vector.tensor_copy(out=xt[kd][:, j * P:(j + 1) * P], in_=tp[:])

                # ---- mm1: ht[m, sel] = gelu(w1^T x_sel^T + b1) -------------------
                for n in range(SEL // 512):
                    for m in range(NM):
                        ph = mm1_psum.tile([P, 512], F32, name="ph")
                        for kd in range(ND):
                            nc.tensor.matmul(
                                out=ph[:],
                                lhsT=w1bf[kd][:, m * P:(m + 1) * P],
                                rhs=xt[kd][:, n * 512:(n + 1) * 512],
                                start=(kd == 0), stop=(kd == ND - 1),
                            )
                        nc.scalar.activation(
                            out=ht[:, m, n * 512:(n + 1) * 512], in_=ph[:],
                            func=Act.Gelu_apprx_tanh, bias=b1t_sb[:, m:m + 1], scale=1.0,
                        )

            # ---- mm2: y[sel, D] = ht^T @ w2 + b2 ---------------------------------
            with ExitStack() as SY:
                y_pool = SY.enter_context(tc.tile_pool(name="y", bufs=2))
                w2_pool = SY.enter_context(tc.tile_pool(name="w2s", bufs=5))
                mm2_psum = SY.enter_context(tc.tile_pool(name="mm2_psum", bufs=8, space="PSUM"))
                for n in range(D // 512):
                    y_sb = y_pool.tile([P, NSJ, 512], BF16, name="y_sb")
                    pys = [mm2_psum.tile([P, 512], F32, name="py") for _ in range(NSJ)]
                    for s in range(NSJ):
                        nc.tensor.matmul(
                            out=pys[s][:], lhsT=o1x128b_sb[:],
                            rhs=b2b_sb[:, n * 512:(n + 1) * 512],
                            start=True, stop=False, skip_group_check=True,
                        )
                    for kg in range(NM // NKGRP):
                        w2t = w2_pool.tile([P, NKGRP, 512], BF16, name="w2t")
                        src = w2b.rearrange("(g p) f -> p g f", p=P)[
                            :, kg * NKGRP:(kg + 1) * NKGRP, n * 512:(n + 1) * 512]
                        nc.gpsimd.dma_start(out=w2t[:], in_=src)
                        for ki in range(NKGRP):
                            kk = kg * NKGRP + ki
                            for s in range(NSJ):
                                nc.tensor.matmul(
                                    out=pys[s][:],
                                    lhsT=ht[:, kk, s * P:(s + 1) * P],
                                    rhs=w2t[:, ki, :],
                                    start=False, stop=(kk == NM - 1),
                                    skip_group_check=True,
                                )
                    for s in range(NSJ):
                        nc.vector.tensor_copy(out=y_sb[:, s, :], in_=pys[s][:])
                    nc.sync.dma_start(
                        out=y_d.rearrange("(j p) d -> p j d", p=P)[
                            :, :, n * 512:(n + 1) * 512],
                        in_=y_sb[:, :, :])

    nc.compile()
    return nc


def make_consts():
    import ml_dtypes
    q = np.arange(P)
    ab = np.zeros((P, 64 * NT), np.float16)
    for c in range(NT):
        ab[:, 64 * c] = q
        ab[:, 64 * c + 32] = 128 * c
    return {
        "abh": ab,
        "ident128": np.eye(P, dtype=np.float32),
        "identb128": np.eye(P, dtype=ml_dtypes.bfloat16),
        "ltri128": (q[:, None] < q[None, :]).astype(np.float32),  # [q, p] = q < p
        "slt32": (np.arange(NT)[:, None] < np.arange(NT)[None, :]).astype(np.float32),
        "id32": np.eye(NT, dtype=np.float32),
        "ones_1x128": np.ones((1, P), np.float32),
        "ones_1x128b": np.ones((1, P), ml_dtypes.bfloat16),
        "ones_128x1": np.ones((P, 1), np.float32),
        "ones_32x128": np.ones((NT, P), np.float32),
    }


def make_in_maps(x, W1, b1, W2, b2, wr, br):
    import ml_dtypes
    consts = make_consts()
    x = np.ascontiguousarray(np.asarray(x, np.float32))
    w1b = np.asarray(W1, np.float32).astype(ml_dtypes.bfloat16)
    w2b = np.asarray(W2, np.float32).astype(ml_dtypes.bfloat16)
    b2b = np.asarray(b2, np.float32).astype(ml_dtypes.bfloat16).reshape(1, D)
    in_maps = []
    for c in range(NCORES):
        b, h = divmod(c, 2)
        m = {
            "x_row": x[b],
            "w1b": w1b,
            "w2b": w2b,
            "wr": np.asarray(wr, np.float32).reshape(1, D),
            "b1t": np.ascontiguousarray(np.asarray(b1, np.float32).reshape(NM, P).T),
            "b2b": b2b,
            "hbase": np.array([[h * SEL]], np.float32),
        }
        m.update(consts)
        in_maps.append(m)
    return in_maps


_NC_CACHE = None


def _get_program():
    global _NC_CACHE
    if _NC_CACHE is None:
        _NC_CACHE = build_program()
    return _NC_CACHE


def kernel(x, W1, b1, W2, b2, wr, br):
    from concourse.bass_utils import run_bass_kernel_spmd

    nc = _get_program()
    in_maps = make_in_maps(x, W1, b1, W2, b2, wr, br)
    res = run_bass_kernel_spmd(nc, in_maps, list(range(NCORES))).results
    out = np.zeros((B, L, D), np.float32)
    for c in range(NCORES):
        b, _h = divmod(c, 2)
        idx = np.asarray(res[c]["sel_d"]).reshape(SEL).astype(np.int64)
        y = np.asarray(res[c]["y_d"]).astype(np.float32)
        out[b, idx] = y
    return out


# revision 22
# speedup vs baseline: 2.5285x; 1.0635x over previous
"""MoD (mixture-of-depths) MLP wrapper kernel for Trainium2, 8 NeuronCores.

Sharding: core c handles batch row b = c//2 and the half of that row's
top-K tokens with global selection ranks in [h*1024, (h+1)*1024), h = c%2.
Each core computes the full row's router scores + top-K threshold locally
(no collectives), inverts rank->token via an fp16 one-hot compaction,
gathers its 1024 token rows (bf16 cast in DMA), runs the FFN in bf16
(fp32 accumulation), and writes a compact result + the token ids.
The host places rows at their token positions while unsharding.

y is produced transposed ([D, SEL]) so mm2 can reuse stationary weights
across the full token width and fuse the output bias per-partition.
"""

import sys

sys.path.insert(0, "/opt/trn_rl_repo")

from contextlib import ExitStack

import numpy as np

from concourse import bass, mybir
from concourse import bacc
import concourse.tile as tile
from concourse.bass import IndirectOffsetOnAxis

B, L, D = 4, 4096, 1024
DFF = 4 * D
K = L // 2              # 2048 selected tokens per row
NCORES = 8
P = 128
NT = L // P             # 32 token tiles per row
SEL = K // 2            # 1024 selected tokens per core
NSJ = SEL // P          # 8 selected-token blocks
ND = D // P             # 8 d chunks
NM = DFF // P           # 32 dff tiles
NKGRP = 4               # w2 k-chunks per streamed tile
RADIX_PASSES = 4
OOB_SENTINEL = 2 * L

F32 = mybir.dt.float32
BF16 = mybir.dt.bfloat16
FP16 = mybir.dt.float16
I32 = mybir.dt.int32
Alu = mybir.AluOpType
Act = mybir.ActivationFunctionType


def build_program():
    nc = bacc.Bacc(
        "TRN2",
        target_bir_lowering=False,
        debug=False,
        enable_asserts=False,
        num_devices=NCORES,
    )

    x_row = nc.dram_tensor("x_row", [L, D], F32, kind="ExternalInput").ap()
    w1b = nc.dram_tensor("w1b", [D, DFF], BF16, kind="ExternalInput").ap()
    w2b = nc.dram_tensor("w2b", [DFF, D], BF16, kind="ExternalInput").ap()
    wr = nc.dram_tensor("wr", [1, D], F32, kind="ExternalInput").ap()
    b1t = nc.dram_tensor("b1t", [P, NM], F32, kind="ExternalInput").ap()
    b2t = nc.dram_tensor("b2t", [P, ND], F32, kind="ExternalInput").ap()
    hbase = nc.dram_tensor("hbase", [1, 1], F32, kind="ExternalInput").ap()
    ident = nc.dram_tensor("ident128", [P, P], F32, kind="ExternalInput").ap()
    identb = nc.dram_tensor("identb128", [P, P], BF16, kind="ExternalInput").ap()
    ltri = nc.dram_tensor("ltri128", [P, P], F32, kind="ExternalInput").ap()
    slt32 = nc.dram_tensor("slt32", [NT, NT], F32, kind="ExternalInput").ap()
    id32 = nc.dram_tensor("id32", [NT, NT], F32, kind="ExternalInput").ap()
    ones_1x128 = nc.dram_tensor("ones_1x128", [1, P], F32, kind="ExternalInput").ap()
    ones_128x1 = nc.dram_tensor("ones_128x1", [P, 1], F32, kind="ExternalInput").ap()
    ones_32x128 = nc.dram_tensor("ones_32x128", [NT, P], F32, kind="ExternalInput").ap()
    abh = nc.dram_tensor("abh", [P, 64 * NT], FP16, kind="ExternalInput").ap()

    y_d = nc.dram_tensor("y_d", [D, SEL], BF16, kind="ExternalOutput").ap()
    sel_d = nc.dram_tensor("sel_d", [SEL, 1], F32, kind="ExternalOutput").ap()

    with tile.TileContext(nc) as tc, ExitStack() as S0:
        const = S0.enter_context(tc.tile_pool(name="const", bufs=1))
        w1_pool = S0.enter_context(tc.tile_pool(name="w1bf", bufs=1))

        def cload(pool, ap, shape, dtype=F32, name=None):
            t = pool.tile(shape, dtype, name=name)
            nc.sync.dma_start(out=t[:], in_=ap)
            return t

        # urgent consts only — everything else queues behind the x stream
        wr_sb = cload(const, wr, [1, D], name="c_wr")
        o1x128_sb = cload(const, ones_1x128, [1, P], name="c_o1")
        ident_sb = cload(const, ident, [P, P], name="c_id")

        iota_i = const.tile([P, 1], I32)
        nc.gpsimd.iota(iota_i[:], pattern=[[1, 1]], base=0, channel_multiplier=1)
        iota_f = const.tile([P, 1], F32)
        nc.vector.tensor_copy(out=iota_f[:], in_=iota_i[:])
        iota1024_i = const.tile([P, SEL], I32)
        nc.gpsimd.iota(iota1024_i[:], pattern=[[1, SEL]], base=0,
                       channel_multiplier=0)
        iota1024h = const.tile([P, SEL], FP16)
        nc.vector.tensor_copy(out=iota1024h[:], in_=iota1024_i[:])

        scores_sb = const.tile([P, NT], F32)
        selidx_sb = const.tile([P, NSJ], I32)

        with ExitStack() as SREP:
            rep_pool = SREP.enter_context(tc.tile_pool(name="rep", bufs=1))
            scores_row = rep_pool.tile([1, L], F32)
            scores_rep = rep_pool.tile([P, L], F32)

            # ---- phase A: router scores (fp32, exact) + replicated scores ------
            with ExitStack() as SA:
                apool = SA.enter_context(tc.tile_pool(name="apool", bufs=1))
                xs_pool = SA.enter_context(tc.tile_pool(name="xs", bufs=5))
                junk_pool = SA.enter_context(tc.tile_pool(name="junk", bufs=2))
                pa_psum = SA.enter_context(tc.tile_pool(name="pa_psum", bufs=2, space="PSUM"))

                wrb = apool.tile([P, D], F32)
                for n in range(D // 512):
                    pt = pa_psum.tile([P, 512], F32, name="pa_mp")
                    nc.tensor.matmul(out=pt[:], lhsT=o1x128_sb[:],
                                     rhs=wr_sb[:, n * 512:(n + 1) * 512],
                                     start=True, stop=True)
                    nc.vector.tensor_copy(out=wrb[:, n * 512:(n + 1) * 512], in_=pt[:])

                PSP = 640        # DVE's share of the score product
                for t in range(NT):
                    x_t = xs_pool.tile([P, D], F32)
                    nc.sync.dma_start(out=x_t[:], in_=x_row[t * P:(t + 1) * P, :])
                    prod = junk_pool.tile([P, D], F32, name="prod")
                    nc.vector.tensor_tensor(out=prod[:, :PSP], in0=x_t[:, :PSP],
                                            in1=wrb[:, :PSP], op=Alu.mult)
                    nc.gpsimd.tensor_tensor(out=prod[:, PSP:], in0=x_t[:, PSP:],
                                            in1=wrb[:, PSP:], op=Alu.mult)
                    sink = junk_pool.tile([P, D], BF16, name="sink")
                    nc.scalar.activation(out=sink[:], in_=prod[:], func=Act.Identity,
                                         bias=0.0, scale=1.0,
                                         accum_out=scores_sb[:, t:t + 1])
                    tpp = pa_psum.tile([1, P], F32, name="pa_tp")
                    nc.tensor.transpose(out=tpp[:], in_=scores_sb[:, t:t + 1],
                                        identity=ident_sb[:])
                    nc.vector.tensor_copy(out=scores_row[:, t * P:(t + 1) * P],
                                          in_=tpp[:])
                    if t % 4 == 3:
                        n = t // 4
                        bp = pa_psum.tile([P, 512], F32, name="pa_mp")
                        nc.tensor.matmul(out=bp[:], lhsT=o1x128_sb[:],
                                         rhs=scores_row[:, n * 512:(n + 1) * 512],
                                         start=True, stop=True)
                        nc.vector.tensor_copy(
                            out=scores_rep[:, n * 512:(n + 1) * 512], in_=bp[:])

            # ---- deferred consts + resident w1 (sync queue, after x) -----------
            b1t_sb = cload(const, b1t, [P, NM], name="c_b1t")
            b2t_sb = cload(const, b2t, [P, ND], name="c_b2t")
            hb_sb = cload(const, hbase, [1, 1], name="c_hb")
            identb_sb = cload(const, identb, [P, P], BF16, name="c_idb")
            ltri_sb = cload(const, ltri, [P, P], name="c_lt")
            slt32_sb = cload(const, slt32, [NT, NT], name="c_sl")
            id32_sb = cload(const, id32, [NT, NT], name="c_id32")
            o128x1_sb = cload(const, ones_128x1, [P, 1], name="c_oc")
            o32x128_sb = cload(const, ones_32x128, [NT, P], name="c_o32")
            abh_sb = cload(const, abh, [P, 64 * NT], FP16, name="c_abh")
            hb_col = const.tile([P, 1], F32)
            nc.gpsimd.partition_broadcast(hb_col[:], hb_sb[:])

            w1bf = []
            for kd in range(ND):
                t_ = w1_pool.tile([P, DFF], BF16, name=f"w1bf_{kd}")
                nc.sync.dma_start(out=t_[:], in_=w1b[kd * P:(kd + 1) * P, :])
                w1bf.append(t_)

            # ---- phase C: top-K threshold, 128-way bisection, Act+DVE split ----
            with ExitStack() as SC:
                radix = SC.enter_context(tc.tile_pool(name="radix", bufs=2))
                rjunk = SC.enter_context(tc.tile_pool(name="rjunk", bufs=2))
                rx_psum = SC.enter_context(tc.tile_pool(name="rx_psum", bufs=1, space="PSUM"))

                ACOLS = 2624     # Act's share of the count scan
                DCOLS = L - ACOLS
                neglo = radix.tile([P, 1], F32, name="neglo")
                nc.vector.memset(neglo[:], 16.0)
                w_cur = 32.0 / P
                for _pass in range(RADIX_PASSES):
                    negthr = radix.tile([P, 1], F32, name="negthr")
                    nc.vector.tensor_scalar(out=negthr[:], in0=iota_f[:],
                                            scalar1=-w_cur, scalar2=neglo[:],
                                            op0=Alu.mult, op1=Alu.add)
                    # Act share: acc1 = sum sign(score - thr) = 2*c1 - ACOLS
                    acc1 = radix.tile([P, 1], F32, name="acc1")
                    sink2 = rjunk.tile([P, ACOLS], BF16, name="sink2")
                    nc.scalar.activation(out=sink2[:], in_=scores_rep[:, :ACOLS],
                                         func=Act.Sign, bias=negthr[:], scale=1.0,
                                         accum_out=acc1[:])
                    # DVE share: indicator then 2x bf16 reduce -> c2
                    c2 = radix.tile([P, 1], F32, name="c2")
                    sinkd = rjunk.tile([P, DCOLS], BF16, name="sinkd")
                    nc.vector.tensor_scalar(out=sinkd[:], in0=scores_rep[:, ACOLS:],
                                            scalar1=negthr[:], scalar2=0.0,
                                            op0=Alu.add, op1=Alu.is_ge)
                    nc.vector.tensor_reduce(out=c2[:], in_=sinkd[:],
                                            axis=mybir.AxisListType.X, op=Alu.add)
                    # count >= K  <=>  acc1 + 2*c2 >= 2K - ACOLS
                    comb = radix.tile([P, 1], F32, name="comb")
                    nc.vector.tensor_scalar(out=comb[:], in0=c2[:], scalar1=2.0,
                                            scalar2=acc1[:], op0=Alu.mult,
                                            op1=Alu.add)
                    sel = radix.tile([P, 1], F32, name="sel")
                    nc.vector.tensor_scalar(out=sel[:], in0=comb[:],
                                            scalar1=float(2 * K - ACOLS),
                                            scalar2=None, op0=Alu.is_ge)
                    s_ps = rx_psum.tile([1, 1], F32, name="s_ps")
                    nc.tensor.matmul(out=s_ps[:], lhsT=sel[:], rhs=o128x1_sb[:],
                                     start=True, stop=True)
                    s_sb = radix.tile([1, 1], F32, name="s_sb")
                    nc.vector.tensor_copy(out=s_sb[:], in_=s_ps[:])
                    bc_ps = rx_psum.tile([P, 1], F32, name="bc_ps")
                    nc.tensor.matmul(out=bc_ps[:], lhsT=o1x128_sb[:], rhs=s_sb[:],
                                     start=True, stop=True)
                    delta = radix.tile([P, 1], F32, name="delta")
                    nc.vector.tensor_scalar(out=delta[:], in0=bc_ps[:],
                                            scalar1=-w_cur, scalar2=w_cur,
                                            op0=Alu.mult, op1=Alu.add)
                    neglo2 = radix.tile([P, 1], F32, name="neglo")
                    nc.vector.tensor_tensor(out=neglo2[:], in0=neglo[:],
                                            in1=delta[:], op=Alu.add)
                    neglo = neglo2
                    w_cur /= P

                T_col = radix.tile([P, 1], F32, name="T_col")
                nc.vector.tensor_scalar(out=T_col[:], in0=neglo[:], scalar1=-1.0,
                                        scalar2=None, op0=Alu.mult)

                # ---- mask, global rank, local window offsets --------------------
                maskf = radix.tile([P, NT], F32, name="maskf")
                nc.vector.tensor_scalar(out=maskf[:], in0=scores_sb[:],
                                        scalar1=T_col[:], scalar2=None,
                                        op0=Alu.is_ge)
                colsum_p = rx_psum.tile([NT, 1], F32, name="cs_ps")
                nc.tensor.matmul(out=colsum_p[:], lhsT=maskf[:], rhs=o128x1_sb[:],
                                 start=True, stop=True)
                colsum = radix.tile([NT, 1], F32, name="colsum")
                nc.vector.tensor_copy(out=colsum[:], in_=colsum_p[:])
                excl_p = rx_psum.tile([NT, 1], F32, name="ex_ps")
                nc.tensor.matmul(out=excl_p[:], lhsT=slt32_sb[:], rhs=colsum[:],
                                 start=True, stop=True)
                excl = radix.tile([NT, 1], F32, name="excl")
                nc.vector.tensor_copy(out=excl[:], in_=excl_p[:])
                diag = radix.tile([NT, NT], F32, name="diag")
                nc.vector.tensor_tensor(out=diag[:], in0=id32_sb[:],
                                        in1=excl[:, :1].to_broadcast([NT, NT]),
                                        op=Alu.mult)
                rank_p = rx_psum.tile([P, NT], F32, name="rank_ps")
                nc.tensor.matmul(out=rank_p[:], lhsT=ltri_sb[:], rhs=maskf[:],
                                 start=True, stop=False, skip_group_check=True)
                nc.tensor.matmul(out=rank_p[:], lhsT=o32x128_sb[:], rhs=diag[:],
                                 start=False, stop=True, skip_group_check=True)

                off = radix.tile([P, NT], F32, name="off")
                nc.vector.tensor_scalar(out=off[:], in0=rank_p[:],
                                        scalar1=hb_col[:], scalar2=None,
                                        op0=Alu.subtract)
                w0 = radix.tile([P, NT], F32, name="w0")
                nc.vector.tensor_scalar(out=w0[:], in0=off[:], scalar1=0.0,
                                        scalar2=None, op0=Alu.is_ge)
                w1m = radix.tile([P, NT], F32, name="w1m")
                nc.vector.tensor_scalar(out=w1m[:], in0=off[:], scalar1=float(SEL),
                                        scalar2=None, op0=Alu.is_lt)
                m2 = radix.tile([P, NT], F32, name="m2")
                nc.vector.tensor_tensor(out=m2[:], in0=w0[:], in1=w1m[:], op=Alu.mult)
                m3 = radix.tile([P, NT], F32, name="m3")
                nc.vector.tensor_tensor(out=m3[:], in0=m2[:], in1=maskf[:], op=Alu.mult)
                t1 = radix.tile([P, NT], F32, name="t1")
                nc.vector.tensor_scalar(out=t1[:], in0=off[:],
                                        scalar1=-float(OOB_SENTINEL),
                                        scalar2=None, op0=Alu.add)
                t2 = radix.tile([P, NT], F32, name="t2")
                nc.vector.tensor_tensor(out=t2[:], in0=t1[:], in1=m3[:], op=Alu.mult)
                offf = radix.tile([P, NT], F32, name="offf")
                nc.vector.tensor_scalar(out=offf[:], in0=t2[:],
                                        scalar1=float(OOB_SENTINEL),
                                        scalar2=None, op0=Alu.add)

                # ---- rank -> token-id inversion: fp16 one-hot compaction --------
                # smat_c[p, r] = (local_rank[p, c] == r). lhsT col 0 = iota_p,
                # col 32 = 128*c (both exact fp16): psum row 0 = p*, row 32 =
                # 128*c*; token id = their sum. Partition starts must be 0/32.
                sel_ps = [rx_psum.tile([33, 512], F32, name=f"selps{n2}")
                          for n2 in range(2)]
                for c in range(NT):
                    smat = rjunk.tile([P, SEL], FP16, name="smat")
                    nc.vector.tensor_scalar(out=smat[:], in0=iota1024h[:],
                                            scalar1=offf[:, c:c + 1], scalar2=None,
                                            op0=Alu.is_equal)
                    for n2 in range(2):
                        nc.tensor.matmul(
                            out=sel_ps[n2][:], lhsT=abh_sb[:, 64 * c:64 * c + 33],
                            rhs=smat[:, n2 * 512:(n2 + 1) * 512],
                            start=(c == 0), stop=(c == NT - 1),
                            skip_group_check=True)
                a_sb = radix.tile([1, SEL], F32, name="a_sb")
                selrow = radix.tile([1, SEL], F32, name="selrow")
                for n2 in range(2):
                    nc.vector.tensor_copy(out=a_sb[:, n2 * 512:(n2 + 1) * 512],
                                          in_=sel_ps[n2][0:1, :])
                for n2 in range(2):
                    nc.vector.tensor_tensor(out=selrow[:, n2 * 512:(n2 + 1) * 512],
                                            in0=a_sb[:, n2 * 512:(n2 + 1) * 512],
                                            in1=sel_ps[n2][32:33, :], op=Alu.add)
                nc.sync.dma_start(out=sel_d, in_=selrow[:])
                self_sb = radix.tile([P, NSJ], F32, name="self_sb")
                nc.sync.dma_start(
                    out=self_sb[:],
                    in_=sel_d.rearrange("(j p) one -> p (j one)", p=P))
                nc.vector.tensor_copy(out=selidx_sb[:], in_=self_sb[:])

        # ---- gather (bf16 cast in DMA) + transpose + MLP -----------------------
        with ExitStack() as SM:
            ht_pool = SM.enter_context(tc.tile_pool(name="ht", bufs=1))
            xt_pool = SM.enter_context(tc.tile_pool(name="xt", bufs=1))
            ht = ht_pool.tile([P, NM, SEL], BF16)
            xt_all = xt_pool.tile([P, ND, SEL], BF16)

            with ExitStack() as SB:
                xsel_pool = SB.enter_context(tc.tile_pool(name="xsel", bufs=4))
                tp_psum = SB.enter_context(tc.tile_pool(name="tp_psum", bufs=2, space="PSUM"))
                for j in range(NSJ):
                    xs = xsel_pool.tile([P, D], BF16, name="xsel")
                    nc.gpsimd.indirect_dma_start(
                        out=xs[:], out_offset=None, in_=x_row,
                        in_offset=IndirectOffsetOnAxis(ap=selidx_sb[:, j:j + 1],
                                                       axis=0))
                    tpbig = tp_psum.tile([P, ND, P], BF16, name="tpbig")
                    for kd in range(ND):
                        nc.tensor.transpose(out=tpbig[:, kd, :],
                                            in_=xs[:, kd * P:(kd + 1) * P],
                                            identity=identb_sb[:])
                    nc.vector.tensor_copy(out=xt_all[:, :, j * P:(j + 1) * P],
                                          in_=tpbig[:, :, :])

            # ---- mm1: ht[m, tok] = gelu(w1^T x_sel^T + b1), stationary w1 ------
            # n innermost so each loaded w1 column block serves both halves
            with ExitStack() as S1:
                mm1_psum = S1.enter_context(tc.tile_pool(name="mm1_psum", bufs=6, space="PSUM"))
                for m in range(NM):
                    phs = [mm1_psum.tile([P, 512], F32, name="ph") for _ in range(2)]
                    for kd in range(ND):
                        for n in range(2):
                            nc.tensor.matmul(
                                out=phs[n][:],
                                lhsT=w1bf[kd][:, m * P:(m + 1) * P],
                                rhs=xt_all[:, kd, n * 512:(n + 1) * 512],
                                start=(kd == 0), stop=(kd == ND - 1),
                                skip_group_check=True,
                            )
                    for n in range(2):
                        nc.scalar.activation(
                            out=ht[:, m, n * 512:(n + 1) * 512], in_=phs[n][:],
                            func=Act.Gelu_apprx_tanh, bias=b1t_sb[:, m:m + 1],
                            scale=1.0,
                        )

            # ---- mm2: y^T[d, tok] = w2^T ht + b2, stationary w2 chunks ---------
            with ExitStack() as SY:
                y_pool = SY.enter_context(tc.tile_pool(name="y", bufs=2))
                w2_pool = SY.enter_context(tc.tile_pool(name="w2s", bufs=5))
                mm2_psum = SY.enter_context(tc.tile_pool(name="mm2_psum", bufs=4, space="PSUM"))
                NDG = 4                      # d-groups of 2*P columns
                DCW = D // NDG               # 256
                for dg in range(NDG):
                    pz = [[mm2_psum.tile([P, 512], F32, name="pz") for _ in range(2)]
                          for _ in range(2)]
                    for kg in range(NM // NKGRP):
                        w2t = w2_pool.tile([P, NKGRP, DCW], BF16, name="w2t")
                        src = w2b.rearrange("(g p) f -> p g f", p=P)[
                            :, kg * NKGRP:(kg + 1) * NKGRP,
                            dg * DCW:(dg + 1) * DCW]
                        nc.gpsimd.dma_start(out=w2t[:], in_=src)
                        for ki in range(NKGRP):
                            kk = kg * NKGRP + ki
                            for dc in range(2):
                                for n in range(2):
                                    nc.tensor.matmul(
                                        out=pz[dc][n][:],
                                        lhsT=w2t[:, ki, dc * P:(dc + 1) * P],
                                        rhs=ht[:, kk, n * 512:(n + 1) * 512],
                                        start=(kk == 0), stop=(kk == NM - 1),
                                        skip_group_check=True,
                                    )
                    for dc in range(2):
                        dd = dg * 2 + dc
                        ysb = y_pool.tile([P, SEL], BF16, name="ysb")
                        for n in range(2):
                            nc.scalar.activation(
                                out=ysb[:, n * 512:(n + 1) * 512], in_=pz[dc][n][:],
                                func=Act.Identity,
                                bias=b2t_sb[:, dd:dd + 1], scale=1.0)
                        nc.sync.dma_start(
                            out=y_d.rearrange("(g p) s -> p g s", p=P)[:, dd, :],
                            in_=ysb[:])

    nc.compile()
    return nc


def make_consts():
    import ml_dtypes
    q = np.arange(P)
    ab = np.zeros((P, 64 * NT), np.float16)
    for c in range(NT):
        ab[:, 64 * c] = q
        ab[:, 64 * c + 32] = 128 * c
    return {
        "abh": ab,
        "ident128": np.eye(P, dtype=np.float32),
        "identb128": np.eye(P, dtype=ml_dtypes.bfloat16),
        "ltri128": (q[:, None] < q[None, :]).astype(np.float32),  # [q, p] = q < p
        "slt32": (np.arange(NT)[:, None] < np.arange(NT)[None, :]).astype(np.float32),
        "id32": np.eye(NT, dtype=np.float32),
        "ones_1x128": np.ones((1, P), np.float32),
        "ones_128x1": np.ones((P, 1), np.float32),
        "ones_32x128": np.ones((NT, P), np.float32),
    }


def make_in_maps(x, W1, b1, W2, b2, wr, br):
    import ml_dtypes
    consts = make_consts()
    x = np.ascontiguousarray(np.asarray(x, np.float32))
    w1b = np.asarray(W1, np.float32).astype(ml_dtypes.bfloat16)
    w2b = np.asarray(W2, np.float32).astype(ml_dtypes.bfloat16)
    in_maps = []
    for c in range(NCORES):
        b, h = divmod(c, 2)
        m = {
            "x_row": x[b],
            "w1b": w1b,
            "w2b": w2b,
            "wr": np.asarray(wr, np.float32).reshape(1, D),
            "b1t": np.ascontiguousarray(np.asarray(b1, np.float32).reshape(NM, P).T),
            "b2t": np.ascontiguousarray(np.asarray(b2, np.float32).reshape(ND, P).T),
            "hbase": np.array([[h * SEL]], np.float32),
        }
        m.update(consts)
        in_maps.append(m)
    return in_maps


_NC_CACHE = None


def _get_program():
    global _NC_CACHE
    if _NC_CACHE is None:
        _NC_CACHE = build_program()
    return _NC_CACHE


def kernel(x, W1, b1, W2, b2, wr, br):
    from concourse.bass_utils import run_bass_kernel_spmd

    nc = _get_program()
    in_maps = make_in_maps(x, W1, b1, W2, b2, wr, br)
    res = run_bass_kernel_spmd(nc, in_maps, list(range(NCORES))).results
    out = np.zeros((B, L, D), np.float32)
    for c in range(NCORES):
        b, _h = divmod(c, 2)
        idx = np.asarray(res[c]["sel_d"]).reshape(SEL).astype(np.int64)
        y = np.asarray(res[c]["y_d"]).astype(np.float32)    # [D, SEL]
        out[b, idx] = y.T
    return out


# revision 26
# speedup vs baseline: 2.5899x; 1.0243x over previous
"""MoD (mixture-of-depths) MLP wrapper kernel for Trainium2, 8 NeuronCores.

Sharding: core c handles batch row b = c//2 and the half of that row's
top-K tokens with global selection ranks in [h*1024, (h+1)*1024), h = c%2.
Each core computes the full row's router scores + top-K threshold locally
(no collectives), inverts rank->token via an fp16 one-hot compaction,
gathers its 1024 token rows (bf16 cast in DMA), runs the FFN in bf16
(fp32 accumulation), and writes a compact result + the token ids.
The host places rows at their token positions while unsharding.

y is produced transposed ([D, SEL]) so mm2 can reuse stationary weights
across the full token width and fuse the output bias per-partition.
"""

import sys

sys.path.insert(0, "/opt/trn_rl_repo")

from contextlib import ExitStack

import numpy as np

from concourse import bass, bass_isa, mybir
from concourse import bacc
import concourse.tile as tile
from concourse.bass import IndirectOffsetOnAxis

B, L, D = 4, 4096, 1024
DFF = 4 * D
K = L // 2              # 2048 selected tokens per row
NCORES = 8
P = 128
NT = L // P             # 32 token tiles per row
SEL = K // 2            # 1024 selected tokens per core
NSJ = SEL // P          # 8 selected-token blocks
ND = D // P             # 8 d chunks
NM = DFF // P           # 32 dff tiles
NKGRP = 4               # w2 k-chunks per streamed tile
RADIX_PASSES = 3
OOB_SENTINEL = 2 * L

F32 = mybir.dt.float32
BF16 = mybir.dt.bfloat16
FP16 = mybir.dt.float16
I32 = mybir.dt.int32
Alu = mybir.AluOpType
Act = mybir.ActivationFunctionType


def build_program():
    nc = bacc.Bacc(
        "TRN2",
        target_bir_lowering=False,
        debug=False,
        enable_asserts=False,
        num_devices=NCORES,
    )

    x_row = nc.dram_tensor("x_row", [L, D], F32, kind="ExternalInput").ap()
    w1b = nc.dram_tensor("w1b", [D, DFF], BF16, kind="ExternalInput").ap()
    w2b = nc.dram_tensor("w2b", [DFF, D], BF16, kind="ExternalInput").ap()
    wr = nc.dram_tensor("wr", [1, D], F32, kind="ExternalInput").ap()
    b1t = nc.dram_tensor("b1t", [P, NM], F32, kind="ExternalInput").ap()
    b2t = nc.dram_tensor("b2t", [P, ND], F32, kind="ExternalInput").ap()
    hbase = nc.dram_tensor("hbase", [1, 1], F32, kind="ExternalInput").ap()
    ident = nc.dram_tensor("ident128", [P, P], F32, kind="ExternalInput").ap()
    identb = nc.dram_tensor("identb128", [P, P], BF16, kind="ExternalInput").ap()
    ltri = nc.dram_tensor("ltri128", [P, P], F32, kind="ExternalInput").ap()
    slt32 = nc.dram_tensor("slt32", [NT, NT], F32, kind="ExternalInput").ap()
    id32 = nc.dram_tensor("id32", [NT, NT], F32, kind="ExternalInput").ap()
    ones_1x128 = nc.dram_tensor("ones_1x128", [1, P], F32, kind="ExternalInput").ap()
    ones_128x1 = nc.dram_tensor("ones_128x1", [P, 1], F32, kind="ExternalInput").ap()
    ones_32x128 = nc.dram_tensor("ones_32x128", [NT, P], F32, kind="ExternalInput").ap()
    abh = nc.dram_tensor("abh", [P, 64 * NT], FP16, kind="ExternalInput").ap()

    y_d = nc.dram_tensor("y_d", [D, SEL], BF16, kind="ExternalOutput").ap()
    sel_d = nc.dram_tensor("sel_d", [SEL, 1], F32, kind="ExternalOutput").ap()

    with tile.TileContext(nc) as tc, ExitStack() as S0:
        const = S0.enter_context(tc.tile_pool(name="const", bufs=1))
        w1_pool = S0.enter_context(tc.tile_pool(name="w1bf", bufs=1))

        def cload(pool, ap, shape, dtype=F32, name=None):
            t = pool.tile(shape, dtype, name=name)
            nc.sync.dma_start(out=t[:], in_=ap)
            return t

        # urgent consts via DVE/Act queues; x tiles lead the SP queue
        wr_sb = const.tile([1, D], F32, name="c_wr")
        nc.scalar.dma_start(out=wr_sb[:], in_=wr)
        o1x128_sb = const.tile([1, P], F32, name="c_o1")
        nc.scalar.dma_start(out=o1x128_sb[:], in_=ones_1x128)
        ident_sb = const.tile([P, P], F32, name="c_id")
        nc.scalar.dma_start(out=ident_sb[:], in_=ident)

        iota_i = const.tile([P, 1], I32)
        nc.gpsimd.iota(iota_i[:], pattern=[[1, 1]], base=0, channel_multiplier=1)
        iota_f = const.tile([P, 1], F32)
        nc.vector.tensor_copy(out=iota_f[:], in_=iota_i[:])
        iota1024_i = const.tile([P, SEL], I32)
        nc.gpsimd.iota(iota1024_i[:], pattern=[[1, SEL]], base=0,
                       channel_multiplier=0)
        iota1024h = const.tile([P, SEL], FP16)
        nc.vector.tensor_copy(out=iota1024h[:], in_=iota1024_i[:])

        scores_sb = const.tile([P, NT], F32)
        selidx_sb = const.tile([P, NSJ], I32)

        with ExitStack() as SREP:
            rep_pool = SREP.enter_context(tc.tile_pool(name="rep", bufs=1))
            scores_row = rep_pool.tile([1, L], F32)
            scores_rep = rep_pool.tile([P, L], F32)

            # ---- phase A: router scores (fp32, exact) + replicated scores ------
            with ExitStack() as SA:
                apool = SA.enter_context(tc.tile_pool(name="apool", bufs=1))
                xs_pool = SA.enter_context(tc.tile_pool(name="xs", bufs=5))
                junk_pool = SA.enter_context(tc.tile_pool(name="junk", bufs=2))
                pa_psum = SA.enter_context(tc.tile_pool(name="pa_psum", bufs=2, space="PSUM"))

                wrb = apool.tile([P, D], F32)
                for n in range(D // 512):
                    pt = pa_psum.tile([P, 512], F32, name="pa_mp")
                    nc.tensor.matmul(out=pt[:], lhsT=o1x128_sb[:],
                                     rhs=wr_sb[:, n * 512:(n + 1) * 512],
                                     start=True, stop=True)
                    nc.vector.tensor_copy(out=wrb[:, n * 512:(n + 1) * 512], in_=pt[:])

                PSP = 640        # DVE's share of the score product
                for t in range(NT):
                    x_t = xs_pool.tile([P, D], F32)
                    nc.sync.dma_start(out=x_t[:], in_=x_row[t * P:(t + 1) * P, :])
                    prod = junk_pool.tile([P, D], F32, name="prod")
                    nc.vector.tensor_tensor(out=prod[:, :PSP], in0=x_t[:, :PSP],
                                            in1=wrb[:, :PSP], op=Alu.mult)
                    nc.gpsimd.tensor_tensor(out=prod[:, PSP:], in0=x_t[:, PSP:],
                                            in1=wrb[:, PSP:], op=Alu.mult)
                    sink = junk_pool.tile([P, D], BF16, name="sink")
                    nc.scalar.activation(out=sink[:], in_=prod[:], func=Act.Identity,
                                         bias=0.0, scale=1.0,
                                         accum_out=scores_sb[:, t:t + 1])
                    tpp = pa_psum.tile([1, P], F32, name="pa_tp")
                    nc.tensor.transpose(out=tpp[:], in_=scores_sb[:, t:t + 1],
                                        identity=ident_sb[:])
                    nc.vector.tensor_copy(out=scores_row[:, t * P:(t + 1) * P],
                                          in_=tpp[:])
                    if t % 4 == 3:
                        n = t // 4
                        bp = pa_psum.tile([P, 512], F32, name="pa_mp")
                        nc.tensor.matmul(out=bp[:], lhsT=o1x128_sb[:],
                                         rhs=scores_row[:, n * 512:(n + 1) * 512],
                                         start=True, stop=True)
                        nc.vector.tensor_copy(
                            out=scores_rep[:, n * 512:(n + 1) * 512], in_=bp[:])

            # ---- deferred consts + resident w1 (sync queue, after x) -----------
            b1t_sb = cload(const, b1t, [P, NM], name="c_b1t")
            b2t_sb = cload(const, b2t, [P, ND], name="c_b2t")
            hb_sb = cload(const, hbase, [1, 1], name="c_hb")
            identb_sb = cload(const, identb, [P, P], BF16, name="c_idb")
            ltri_sb = cload(const, ltri, [P, P], name="c_lt")
            slt32_sb = cload(const, slt32, [NT, NT], name="c_sl")
            id32_sb = cload(const, id32, [NT, NT], name="c_id32")
            o128x1_sb = cload(const, ones_128x1, [P, 1], name="c_oc")
            o32x128_sb = cload(const, ones_32x128, [NT, P], name="c_o32")
            abh_sb = cload(const, abh, [P, 64 * NT], FP16, name="c_abh")
            hb_col = const.tile([P, 1], F32)
            nc.gpsimd.partition_broadcast(hb_col[:], hb_sb[:])

            w1bf = []
            for kd in range(ND):
                t_ = w1_pool.tile([P, DFF], BF16, name=f"w1bf_{kd}")
                nc.sync.dma_start(out=t_[:], in_=w1b[kd * P:(kd + 1) * P, :])
                w1bf.append(t_)

            # ---- phase C: top-K threshold, 128-way bisection, Act+DVE split ----
            with ExitStack() as SC:
                radix = SC.enter_context(tc.tile_pool(name="radix", bufs=2))
                rjunk = SC.enter_context(tc.tile_pool(name="rjunk", bufs=2))
                rx_psum = SC.enter_context(tc.tile_pool(name="rx_psum", bufs=1, space="PSUM"))

                ACOLS = 2624     # Act's share of the count scan
                DCOLS = L - ACOLS
                neglo = radix.tile([P, 1], F32, name="neglo")
                nc.vector.memset(neglo[:], 16.0)
                w_cur = 32.0 / P
                for _pass in range(RADIX_PASSES):
                    negthr = radix.tile([P, 1], F32, name="negthr")
                    nc.vector.tensor_scalar(out=negthr[:], in0=iota_f[:],
                                            scalar1=-w_cur, scalar2=neglo[:],
                                            op0=Alu.mult, op1=Alu.add)
                    # Act share: acc1 = sum sign(score - thr) = 2*c1 - ACOLS
                    acc1 = radix.tile([P, 1], F32, name="acc1")
                    sink2 = rjunk.tile([P, ACOLS], BF16, name="sink2")
                    nc.scalar.activation(out=sink2[:], in_=scores_rep[:, :ACOLS],
                                         func=Act.Sign, bias=negthr[:], scale=1.0,
                                         accum_out=acc1[:])
                    # DVE share: indicator then 2x bf16 reduce -> c2
                    c2 = radix.tile([P, 1], F32, name="c2")
                    sinkd = rjunk.tile([P, DCOLS], BF16, name="sinkd")
                    nc.vector.tensor_scalar(out=sinkd[:], in0=scores_rep[:, ACOLS:],
                                            scalar1=negthr[:], scalar2=0.0,
                                            op0=Alu.add, op1=Alu.is_ge)
                    nc.vector.tensor_reduce(out=c2[:], in_=sinkd[:],
                                            axis=mybir.AxisListType.X, op=Alu.add)
                    # count >= K  <=>  acc1 + 2*c2 >= 2K - ACOLS
                    comb = radix.tile([P, 1], F32, name="comb")
                    nc.vector.tensor_scalar(out=comb[:], in0=c2[:], scalar1=2.0,
                                            scalar2=acc1[:], op0=Alu.mult,
                                            op1=Alu.add)
                    sel = radix.tile([P, 1], F32, name="sel")
                    nc.vector.tensor_scalar(out=sel[:], in0=comb[:],
                                            scalar1=float(2 * K - ACOLS),
                                            scalar2=None, op0=Alu.is_ge)
                    s_col = radix.tile([P, 1], F32, name="s_col")
                    nc.gpsimd.partition_all_reduce(s_col[:], sel[:], channels=P,
                                                   reduce_op=bass_isa.ReduceOp.add)
                    delta = radix.tile([P, 1], F32, name="delta")
                    nc.vector.tensor_scalar(out=delta[:], in0=s_col[:],
                                            scalar1=-w_cur, scalar2=w_cur,
                                            op0=Alu.mult, op1=Alu.add)
                    neglo2 = radix.tile([P, 1], F32, name="neglo")
                    nc.vector.tensor_tensor(out=neglo2[:], in0=neglo[:],
                                            in1=delta[:], op=Alu.add)
                    neglo = neglo2
                    w_cur /= P

                T_col = radix.tile([P, 1], F32, name="T_col")
                nc.vector.tensor_scalar(out=T_col[:], in0=neglo[:], scalar1=-1.0,
                                        scalar2=None, op0=Alu.mult)

                # ---- mask, global rank, local window offsets --------------------
                maskf = radix.tile([P, NT], F32, name="maskf")
                nc.vector.tensor_scalar(out=maskf[:], in0=scores_sb[:],
                                        scalar1=T_col[:], scalar2=None,
                                        op0=Alu.is_ge)
                colsum_p = rx_psum.tile([NT, 1], F32, name="cs_ps")
                nc.tensor.matmul(out=colsum_p[:], lhsT=maskf[:], rhs=o128x1_sb[:],
                                 start=True, stop=True)
                colsum = radix.tile([NT, 1], F32, name="colsum")
                nc.vector.tensor_copy(out=colsum[:], in_=colsum_p[:])
                excl_p = rx_psum.tile([NT, 1], F32, name="ex_ps")
                nc.tensor.matmul(out=excl_p[:], lhsT=slt32_sb[:], rhs=colsum[:],
                                 start=True, stop=True)
                excl = radix.tile([NT, 1], F32, name="excl")
                nc.vector.tensor_copy(out=excl[:], in_=excl_p[:])
                diag = radix.tile([NT, NT], F32, name="diag")
                nc.vector.tensor_tensor(out=diag[:], in0=id32_sb[:],
                                        in1=excl[:, :1].to_broadcast([NT, NT]),
                                        op=Alu.mult)
                rank_p = rx_psum.tile([P, NT], F32, name="rank_ps")
                nc.tensor.matmul(out=rank_p[:], lhsT=ltri_sb[:], rhs=maskf[:],
                                 start=True, stop=False, skip_group_check=True)
                nc.tensor.matmul(out=rank_p[:], lhsT=o32x128_sb[:], rhs=diag[:],
                                 start=False, stop=True, skip_group_check=True)

                off = radix.tile([P, NT], F32, name="off")
                nc.vector.tensor_scalar(out=off[:], in0=rank_p[:],
                                        scalar1=hb_col[:], scalar2=None,
                                        op0=Alu.subtract)
                w0 = radix.tile([P, NT], F32, name="w0")
                nc.vector.tensor_scalar(out=w0[:], in0=off[:], scalar1=0.0,
                                        scalar2=None, op0=Alu.is_ge)
                w1m = radix.tile([P, NT], F32, name="w1m")
                nc.vector.tensor_scalar(out=w1m[:], in0=off[:], scalar1=float(SEL),
                                        scalar2=None, op0=Alu.is_lt)
                m2 = radix.tile([P, NT], F32, name="m2")
                nc.vector.tensor_tensor(out=m2[:], in0=w0[:], in1=w1m[:], op=Alu.mult)
                m3 = radix.tile([P, NT], F32, name="m3")
                nc.vector.tensor_tensor(out=m3[:], in0=m2[:], in1=maskf[:], op=Alu.mult)
                t1 = radix.tile([P, NT], F32, name="t1")
                nc.vector.tensor_scalar(out=t1[:], in0=off[:],
                                        scalar1=-float(OOB_SENTINEL),
                                        scalar2=None, op0=Alu.add)
                t2 = radix.tile([P, NT], F32, name="t2")
                nc.vector.tensor_tensor(out=t2[:], in0=t1[:], in1=m3[:], op=Alu.mult)
                offf = radix.tile([P, NT], F32, name="offf")
                nc.vector.tensor_scalar(out=offf[:], in0=t2[:],
                                        scalar1=float(OOB_SENTINEL),
                                        scalar2=None, op0=Alu.add)

                # ---- rank -> token-id inversion: fp16 one-hot compaction --------
                # smat_c[p, r] = (local_rank[p, c] == r). lhsT col 0 = iota_p,
                # col 32 = 128*c (both exact fp16): psum row 0 = p*, row 32 =
                # 128*c*; token id = their sum. Partition starts must be 0/32.
                sel_ps = [rx_psum.tile([33, 512], F32, name=f"selps{n2}")
                          for n2 in range(2)]
                for c in range(NT):
                    smat = rjunk.tile([P, SEL], FP16, name="smat")
                    nc.vector.tensor_scalar(out=smat[:], in0=iota1024h[:],
                                            scalar1=offf[:, c:c + 1], scalar2=None,
                                            op0=Alu.is_equal)
                    for n2 in range(2):
                        nc.tensor.matmul(
                            out=sel_ps[n2][:], lhsT=abh_sb[:, 64 * c:64 * c + 33],
                            rhs=smat[:, n2 * 512:(n2 + 1) * 512],
                            start=(c == 0), stop=(c == NT - 1),
                            skip_group_check=True)
                a_sb = radix.tile([1, SEL], F32, name="a_sb")
                selrow = radix.tile([1, SEL], F32, name="selrow")
                sidx_ps = rx_psum.tile([P, NSJ], F32, name="sidx_ps")
                for n2 in range(2):
                    nc.vector.tensor_copy(out=a_sb[:, n2 * 512:(n2 + 1) * 512],
                                          in_=sel_ps[n2][0:1, :])
                    nc.vector.tensor_tensor(out=selrow[:, n2 * 512:(n2 + 1) * 512],
                                            in0=a_sb[:, n2 * 512:(n2 + 1) * 512],
                                            in1=sel_ps[n2][32:33, :], op=Alu.add)
                    # transpose each 128-chunk to a psum column: trivial matmul
                    # with a [1,1] ones rhs maps selrow[j*128+p] -> [p, j]
                    for jj in range(4):
                        j = n2 * 4 + jj
                        nc.tensor.matmul(
                            out=sidx_ps[:, j:j + 1],
                            lhsT=selrow[:, j * P:(j + 1) * P],
                            rhs=o1x128_sb[:, 0:1],
                            start=True, stop=True, skip_group_check=True)
                nc.vector.tensor_copy(out=selidx_sb[:], in_=sidx_ps[:])
                # host-visible token ids; not on the gather critical path
                nc.sync.dma_start(out=sel_d, in_=selrow[:])

        # ---- gather (bf16 cast in DMA) + transpose + MLP -----------------------
        with ExitStack() as SM:
            ht_pool = SM.enter_context(tc.tile_pool(name="ht", bufs=1))
            xt_pool = SM.enter_context(tc.tile_pool(name="xt", bufs=1))
            ht = ht_pool.tile([P, NM, SEL], BF16)
            xt_all = xt_pool.tile([P, ND, SEL], BF16)

            with ExitStack() as SB:
                xsel_pool = SB.enter_context(tc.tile_pool(name="xsel", bufs=4))
                tp_psum = SB.enter_context(tc.tile_pool(name="tp_psum", bufs=2, space="PSUM"))
                for j in range(NSJ):
                    xs = xsel_pool.tile([P, D], BF16, name="xsel")
                    nc.gpsimd.indirect_dma_start(
                        out=xs[:], out_offset=None, in_=x_row,
                        in_offset=IndirectOffsetOnAxis(ap=selidx_sb[:, j:j + 1],
                                                       axis=0))
                    tpbig = tp_psum.tile([P, ND, P], BF16, name="tpbig")
                    for kd in range(ND):
                        nc.tensor.transpose(out=tpbig[:, kd, :],
                                            in_=xs[:, kd * P:(kd + 1) * P],
                                            identity=identb_sb[:])
                    nc.vector.tensor_copy(out=xt_all[:, :, j * P:(j + 1) * P],
                                          in_=tpbig[:, :, :])

            # ---- mm1: ht[m, tok] = gelu(w1^T x_sel^T + b1) ---------------------
            # n outer: the first token half only needs gather blocks j=0..3
            with ExitStack() as S1:
                mm1_psum = S1.enter_context(tc.tile_pool(name="mm1_psum", bufs=6, space="PSUM"))
                for n in range(2):
                    for m in range(NM):
                        ph = mm1_psum.tile([P, 512], F32, name="ph")
                        for kd in range(ND):
                            nc.tensor.matmul(
                                out=ph[:],
                                lhsT=w1bf[kd][:, m * P:(m + 1) * P],
                                rhs=xt_all[:, kd, n * 512:(n + 1) * 512],
                                start=(kd == 0), stop=(kd == ND - 1),
                            )
                        nc.scalar.activation(
                            out=ht[:, m, n * 512:(n + 1) * 512], in_=ph[:],
                            func=Act.Gelu_apprx_tanh, bias=b1t_sb[:, m:m + 1],
                            scale=1.0,
                        )

            # ---- mm2: y^T[d, tok] = w2^T ht + b2, stationary w2 chunks ---------
            with ExitStack() as SY:
                y_pool = SY.enter_context(tc.tile_pool(name="y", bufs=2))
                w2_pool = SY.enter_context(tc.tile_pool(name="w2s", bufs=5))
                mm2_psum = SY.enter_context(tc.tile_pool(name="mm2_psum", bufs=4, space="PSUM"))
                NDG = 4                      # d-groups of 2*P columns
                DCW = D // NDG               # 256
                for dg in range(NDG):
                    pz = [[mm2_psum.tile([P, 512], F32, name="pz") for _ in range(2)]
                          for _ in range(2)]
                    for kg in range(NM // NKGRP):
                        w2t = w2_pool.tile([P, NKGRP, DCW], BF16, name="w2t")
                        src = w2b.rearrange("(g p) f -> p g f", p=P)[
                            :, kg * NKGRP:(kg + 1) * NKGRP,
                            dg * DCW:(dg + 1) * DCW]
                        nc.gpsimd.dma_start(out=w2t[:], in_=src)
                        for ki in range(NKGRP):
                            kk = kg * NKGRP + ki
                            for dc in range(2):
                                for n in range(2):
                                    nc.tensor.matmul(
                                        out=pz[dc][n][:],
                                        lhsT=w2t[:, ki, dc * P:(dc + 1) * P],
                                        rhs=ht[:, kk, n * 512:(n + 1) * 512],
                                        start=(kk == 0), stop=(kk == NM - 1),
                                        skip_group_check=True,
                                    )
                    for dc in range(2):
                        dd = dg * 2 + dc
                        ysb = y_pool.tile([P, SEL], BF16, name="ysb")
                        nc.scalar.activation(
                            out=ysb[:, 0:512], in_=pz[dc][0][:],
                            func=Act.Identity,
                            bias=b2t_sb[:, dd:dd + 1], scale=1.0)
                        nc.vector.tensor_scalar(
                            out=ysb[:, 512:1024], in0=pz[dc][1][:],
                            scalar1=b2t_sb[:, dd:dd + 1], scalar2=None,
                            op0=Alu.add)
                        nc.sync.dma_start(
                            out=y_d.rearrange("(g p) s -> p g s", p=P)[:, dd, :],
                            in_=ysb[:])

    nc.compile()
    return nc


def make_consts():
    import ml_dtypes
    q = np.arange(P)
    ab = np.zeros((P, 64 * NT), np.float16)
    for c in range(NT):
        ab[:, 64 * c] = q
        ab[:, 64 * c + 32] = 128 * c
    return {
        "abh": ab,
        "ident128": np.eye(P, dtype=np.float32),
        "identb128": np.eye(P, dtype=ml_dtypes.bfloat16),
        "ltri128": (q[:, None] < q[None, :]).astype(np.float32),  # [q, p] = q < p
        "slt32": (np.arange(NT)[:, None] < np.arange(NT)[None, :]).astype(np.float32),
        "id32": np.eye(NT, dtype=np.float32),
        "ones_1x128": np.ones((1, P), np.float32),
        "ones_128x1": np.ones((P, 1), np.float32),
        "ones_32x128": np.ones((NT, P), np.float32),
    }


def make_in_maps(x, W1, b1, W2, b2, wr, br):
    import ml_dtypes
    consts = make_consts()
    x = np.ascontiguousarray(np.asarray(x, np.float32))
    w1b = np.asarray(W1, np.float32).astype(ml_dtypes.bfloat16)
    w2b = np.asarray(W2, np.float32).astype(ml_dtypes.bfloat16)
    in_maps = []
    for c in range(NCORES):
        b, h = divmod(c, 2)
        m = {
            "x_row": x[b],
            "w1b": w1b,
            "w2b": w2b,
            "wr": np.asarray(wr, np.float32).reshape(1, D),
            "b1t": np.ascontiguousarray(np.asarray(b1, np.float32).reshape(NM, P).T),
            "b2t": np.ascontiguousarray(np.asarray(b2, np.float32).reshape(ND, P).T),
            "hbase": np.array([[h * SEL]], np.float32),
        }
        m.update(consts)
        in_maps.append(m)
    return in_maps


_NC_CACHE = None


def _get_program():
    global _NC_CACHE
    if _NC_CACHE is None:
        _NC_CACHE = build_program()
    return _NC_CACHE


def kernel(x, W1, b1, W2, b2, wr, br):
    from concourse.bass_utils import run_bass_kernel_spmd

    nc = _get_program()
    in_maps = make_in_maps(x, W1, b1, W2, b2, wr, br)
    res = run_bass_kernel_spmd(nc, in_maps, list(range(NCORES))).results
    out = np.zeros((B, L, D), np.float32)
    for c in range(NCORES):
        b, _h = divmod(c, 2)
        idx = np.asarray(res[c]["sel_d"]).reshape(SEL).astype(np.int64)
        y = np.asarray(res[c]["y_d"]).astype(np.float32)    # [D, SEL]
        out[b, idx] = y.T
    return out


# revision 28
# speedup vs baseline: 2.6717x; 1.0316x over previous
"""MoD (mixture-of-depths) MLP wrapper kernel for Trainium2, 8 NeuronCores.

Sharding: core c handles batch row b = c//2 and the half of that row's
top-K tokens with global selection ranks in [h*1024, (h+1)*1024), h = c%2.
Each core computes the full row's router scores + top-K threshold locally
(no collectives), inverts rank->token via an fp16 one-hot compaction,
gathers its 1024 token rows (bf16 cast in DMA), runs the FFN in bf16
(fp32 accumulation), and writes a compact result + the token ids.
The host places rows at their token positions while unsharding.

y is produced transposed ([D, SEL]) so mm2 can reuse stationary weights
across the full token width and fuse the output bias per-partition.
"""

import sys

sys.path.insert(0, "/opt/trn_rl_repo")

from contextlib import ExitStack

import numpy as np

from concourse import bass, bass_isa, mybir
from concourse import bacc
import concourse.tile as tile
from concourse.bass import IndirectOffsetOnAxis

B, L, D = 4, 4096, 1024
DFF = 4 * D
K = L // 2              # 2048 selected tokens per row
NCORES = 8
P = 128
NT = L // P             # 32 token tiles per row
SEL = K // 2            # 1024 selected tokens per core
NSJ = SEL // P          # 8 selected-token blocks
ND = D // P             # 8 d chunks
NM = DFF // P           # 32 dff tiles
NKGRP = 4               # w2 k-chunks per streamed tile
RADIX_PASSES = 3
OOB_SENTINEL = 2 * L

F32 = mybir.dt.float32
BF16 = mybir.dt.bfloat16
FP16 = mybir.dt.float16
I32 = mybir.dt.int32
Alu = mybir.AluOpType
Act = mybir.ActivationFunctionType


def build_program():
    nc = bacc.Bacc(
        "TRN2",
        target_bir_lowering=False,
        debug=False,
        enable_asserts=False,
        num_devices=NCORES,
    )

    x_row = nc.dram_tensor("x_row", [L, D], F32, kind="ExternalInput").ap()
    w1b = nc.dram_tensor("w1b", [D, DFF], BF16, kind="ExternalInput").ap()
    w2b = nc.dram_tensor("w2b", [DFF, D], BF16, kind="ExternalInput").ap()
    wr = nc.dram_tensor("wr", [1, D], F32, kind="ExternalInput").ap()
    b1t = nc.dram_tensor("b1t", [P, NM], F32, kind="ExternalInput").ap()
    b2t = nc.dram_tensor("b2t", [P, ND], F32, kind="ExternalInput").ap()
    hbase = nc.dram_tensor("hbase", [1, 1], F32, kind="ExternalInput").ap()
    ident = nc.dram_tensor("ident128", [P, P], F32, kind="ExternalInput").ap()
    identb = nc.dram_tensor("identb128", [P, P], BF16, kind="ExternalInput").ap()
    ltri = nc.dram_tensor("ltri128", [P, P], F32, kind="ExternalInput").ap()
    slt32 = nc.dram_tensor("slt32", [NT, NT], F32, kind="ExternalInput").ap()
    id32 = nc.dram_tensor("id32", [NT, NT], F32, kind="ExternalInput").ap()
    ones_1x128 = nc.dram_tensor("ones_1x128", [1, P], F32, kind="ExternalInput").ap()
    ones_128x1 = nc.dram_tensor("ones_128x1", [P, 1], F32, kind="ExternalInput").ap()
    ones_32x128 = nc.dram_tensor("ones_32x128", [NT, P], F32, kind="ExternalInput").ap()
    j128a = nc.dram_tensor("j128a", [P, NT, NSJ], F32, kind="ExternalInput").ap()
    j128b = nc.dram_tensor("j128b", [P, NT, NSJ], F32, kind="ExternalInput").ap()
    jvals = nc.dram_tensor("jvals", [P, NT, NSJ], F32, kind="ExternalInput").ap()
    lowf = nc.dram_tensor("lowf", [P, NT], F32, kind="ExternalInput").ap()
    i128h = nc.dram_tensor("i128h", [P, P], FP16, kind="ExternalInput").ap()

    y_d = nc.dram_tensor("y_d", [D, SEL], BF16, kind="ExternalOutput").ap()
    sel_d = nc.dram_tensor("sel_d", [SEL, 1], F32, kind="ExternalOutput").ap()

    with tile.TileContext(nc) as tc, ExitStack() as S0:
        const = S0.enter_context(tc.tile_pool(name="const", bufs=1))
        w1_pool = S0.enter_context(tc.tile_pool(name="w1bf", bufs=1))

        def cload(pool, ap, shape, dtype=F32, name=None):
            t = pool.tile(shape, dtype, name=name)
            nc.sync.dma_start(out=t[:], in_=ap)
            return t

        # urgent consts via DVE/Act queues; x tiles lead the SP queue
        wr_sb = const.tile([1, D], F32, name="c_wr")
        nc.gpsimd.dma_start(out=wr_sb[:], in_=wr)
        o1x128_sb = const.tile([1, P], F32, name="c_o1")
        nc.gpsimd.dma_start(out=o1x128_sb[:], in_=ones_1x128)
        ident_sb = const.tile([P, P], F32, name="c_id")
        nc.scalar.dma_start(out=ident_sb[:], in_=ident)

        iota_i = const.tile([P, 1], I32)
        nc.gpsimd.iota(iota_i[:], pattern=[[1, 1]], base=0, channel_multiplier=1)
        iota_f = const.tile([P, 1], F32)
        nc.vector.tensor_copy(out=iota_f[:], in_=iota_i[:])


        scores_sb = const.tile([P, NT], F32)
        selidx_sb = const.tile([P, NSJ], I32)

        with ExitStack() as SREP:
            rep_pool = SREP.enter_context(tc.tile_pool(name="rep", bufs=1))
            scores_row = rep_pool.tile([1, L], F32)
            scores_rep = rep_pool.tile([P, L], F32)

            # ---- phase A: router scores (fp32, exact) + replicated scores ------
            with ExitStack() as SA:
                apool = SA.enter_context(tc.tile_pool(name="apool", bufs=1))
                xs_pool = SA.enter_context(tc.tile_pool(name="xs", bufs=8))
                junk_pool = SA.enter_context(tc.tile_pool(name="junk", bufs=2))
                pa_psum = SA.enter_context(tc.tile_pool(name="pa_psum", bufs=2, space="PSUM"))

                wrb = apool.tile([P, D], F32)
                for n in range(D // 512):
                    pt = pa_psum.tile([P, 512], F32, name="pa_mp")
                    nc.tensor.matmul(out=pt[:], lhsT=o1x128_sb[:],
                                     rhs=wr_sb[:, n * 512:(n + 1) * 512],
                                     start=True, stop=True)
                    nc.vector.tensor_copy(out=wrb[:, n * 512:(n + 1) * 512], in_=pt[:])

                PSP = 640        # DVE's share of the score product
                for t in range(NT):
                    x_t = xs_pool.tile([P, D], F32)
                    nc.sync.dma_start(out=x_t[:], in_=x_row[t * P:(t + 1) * P, :])
                    prod = junk_pool.tile([P, D], F32, name="prod")
                    nc.vector.tensor_tensor(out=prod[:, :PSP], in0=x_t[:, :PSP],
                                            in1=wrb[:, :PSP], op=Alu.mult)
                    nc.gpsimd.tensor_tensor(out=prod[:, PSP:], in0=x_t[:, PSP:],
                                            in1=wrb[:, PSP:], op=Alu.mult)
                    sink = junk_pool.tile([P, D], BF16, name="sink")
                    nc.scalar.activation(out=sink[:], in_=prod[:], func=Act.Identity,
                                         bias=0.0, scale=1.0,
                                         accum_out=scores_sb[:, t:t + 1])
                    tpp = pa_psum.tile([1, P], F32, name="pa_tp")
                    nc.tensor.transpose(out=tpp[:], in_=scores_sb[:, t:t + 1],
                                        identity=ident_sb[:])
                    nc.vector.tensor_copy(out=scores_row[:, t * P:(t + 1) * P],
                                          in_=tpp[:])
                    if t % 4 == 3:
                        n = t // 4
                        bp = pa_psum.tile([P, 512], F32, name="pa_mp")
                        nc.tensor.matmul(out=bp[:], lhsT=o1x128_sb[:],
                                         rhs=scores_row[:, n * 512:(n + 1) * 512],
                                         start=True, stop=True)
                        nc.vector.tensor_copy(
                            out=scores_rep[:, n * 512:(n + 1) * 512], in_=bp[:])

            # ---- deferred consts + resident w1 (sync queue, after x) -----------
            b1t_sb = cload(const, b1t, [P, NM], name="c_b1t")
            b2t_sb = cload(const, b2t, [P, ND], name="c_b2t")
            hb_sb = cload(const, hbase, [1, 1], name="c_hb")
            identb_sb = cload(const, identb, [P, P], BF16, name="c_idb")
            ltri_sb = cload(const, ltri, [P, P], name="c_lt")
            slt32_sb = cload(const, slt32, [NT, NT], name="c_sl")
            id32_sb = cload(const, id32, [NT, NT], name="c_id32")
            o128x1_sb = cload(const, ones_128x1, [P, 1], name="c_oc")
            o32x128_sb = cload(const, ones_32x128, [NT, P], name="c_o32")
            j128a_sb = cload(const, j128a, [P, NT, NSJ], name="c_j128a")
            j128b_sb = cload(const, j128b, [P, NT, NSJ], name="c_j128b")
            jvals_sb = cload(const, jvals, [P, NT, NSJ], name="c_jvals")
            lowf_sb = cload(const, lowf, [P, NT], name="c_lowf")
            i128h_sb = cload(const, i128h, [P, P], FP16, name="c_i128h")
            hb_col = const.tile([P, 1], F32)
            nc.gpsimd.partition_broadcast(hb_col[:], hb_sb[:])

            w1bf = []
            for kd in range(ND):
                t_ = w1_pool.tile([P, DFF], BF16, name=f"w1bf_{kd}")
                nc.sync.dma_start(out=t_[:], in_=w1b[kd * P:(kd + 1) * P, :])
                w1bf.append(t_)

            # ---- phase C: top-K threshold, 128-way bisection, Act+DVE split ----
            with ExitStack() as SC:
                radix = SC.enter_context(tc.tile_pool(name="radix", bufs=2))
                rjunk = SC.enter_context(tc.tile_pool(name="rjunk", bufs=2))
                rx_psum = SC.enter_context(tc.tile_pool(name="rx_psum", bufs=1, space="PSUM"))

                ACOLS = 2624     # Act's share of the count scan
                DCOLS = L - ACOLS
                neglo = radix.tile([P, 1], F32, name="neglo")
                nc.vector.memset(neglo[:], 16.0)
                w_cur = 32.0 / P
                for _pass in range(RADIX_PASSES):
                    negthr = radix.tile([P, 1], F32, name="negthr")
                    nc.vector.tensor_scalar(out=negthr[:], in0=iota_f[:],
                                            scalar1=-w_cur, scalar2=neglo[:],
                                            op0=Alu.mult, op1=Alu.add)
                    # Act share: acc1 = sum sign(score - thr) = 2*c1 - ACOLS
                    acc1 = radix.tile([P, 1], F32, name="acc1")
                    sink2 = rjunk.tile([P, ACOLS], BF16, name="sink2")
                    nc.scalar.activation(out=sink2[:], in_=scores_rep[:, :ACOLS],
                                         func=Act.Sign, bias=negthr[:], scale=1.0,
                                         accum_out=acc1[:])
                    # DVE share: indicator then 2x bf16 reduce -> c2
                    c2 = radix.tile([P, 1], F32, name="c2")
                    sinkd = rjunk.tile([P, DCOLS], BF16, name="sinkd")
                    nc.vector.tensor_scalar(out=sinkd[:], in0=scores_rep[:, ACOLS:],
                                            scalar1=negthr[:], scalar2=0.0,
                                            op0=Alu.add, op1=Alu.is_ge)
                    nc.vector.tensor_reduce(out=c2[:], in_=sinkd[:],
                                            axis=mybir.AxisListType.X, op=Alu.add)
                    # count >= K  <=>  acc1 + 2*c2 >= 2K - ACOLS
                    comb = radix.tile([P, 1], F32, name="comb")
                    nc.vector.tensor_scalar(out=comb[:], in0=c2[:], scalar1=2.0,
                                            scalar2=acc1[:], op0=Alu.mult,
                                            op1=Alu.add)
                    sel = radix.tile([P, 1], F32, name="sel")
                    nc.vector.tensor_scalar(out=sel[:], in0=comb[:],
                                            scalar1=float(2 * K - ACOLS),
                                            scalar2=None, op0=Alu.is_ge)
                    s_col = radix.tile([P, 1], F32, name="s_col")
                    nc.gpsimd.partition_all_reduce(s_col[:], sel[:], channels=P,
                                                   reduce_op=bass_isa.ReduceOp.add)
                    delta = radix.tile([P, 1], F32, name="delta")
                    nc.vector.tensor_scalar(out=delta[:], in0=s_col[:],
                                            scalar1=-w_cur, scalar2=w_cur,
                                            op0=Alu.mult, op1=Alu.add)
                    neglo2 = radix.tile([P, 1], F32, name="neglo")
                    nc.vector.tensor_tensor(out=neglo2[:], in0=neglo[:],
                                            in1=delta[:], op=Alu.add)
                    neglo = neglo2
                    w_cur /= P

                T_col = radix.tile([P, 1], F32, name="T_col")
                nc.vector.tensor_scalar(out=T_col[:], in0=neglo[:], scalar1=-1.0,
                                        scalar2=None, op0=Alu.mult)

                # ---- mask, global rank, local window offsets --------------------
                maskf = radix.tile([P, NT], F32, name="maskf")
                nc.vector.tensor_scalar(out=maskf[:], in0=scores_sb[:],
                                        scalar1=T_col[:], scalar2=None,
                                        op0=Alu.is_ge)
                colsum_p = rx_psum.tile([NT, 1], F32, name="cs_ps")
                nc.tensor.matmul(out=colsum_p[:], lhsT=maskf[:], rhs=o128x1_sb[:],
                                 start=True, stop=True)
                colsum = radix.tile([NT, 1], F32, name="colsum")
                nc.vector.tensor_copy(out=colsum[:], in_=colsum_p[:])
                excl_p = rx_psum.tile([NT, 1], F32, name="ex_ps")
                nc.tensor.matmul(out=excl_p[:], lhsT=slt32_sb[:], rhs=colsum[:],
                                 start=True, stop=True)
                excl = radix.tile([NT, 1], F32, name="excl")
                nc.vector.tensor_copy(out=excl[:], in_=excl_p[:])
                diag = radix.tile([NT, NT], F32, name="diag")
                nc.vector.tensor_tensor(out=diag[:], in0=id32_sb[:],
                                        in1=excl[:, :1].to_broadcast([NT, NT]),
                                        op=Alu.mult)
                rank_p = rx_psum.tile([P, NT], F32, name="rank_ps")
                nc.tensor.matmul(out=rank_p[:], lhsT=ltri_sb[:], rhs=maskf[:],
                                 start=True, stop=False, skip_group_check=True)
                nc.tensor.matmul(out=rank_p[:], lhsT=o32x128_sb[:], rhs=diag[:],
                                 start=False, stop=True, skip_group_check=True)

                off = radix.tile([P, NT], F32, name="off")
                nc.vector.tensor_scalar(out=off[:], in0=rank_p[:],
                                        scalar1=hb_col[:], scalar2=None,
                                        op0=Alu.subtract)
                w0 = radix.tile([P, NT], F32, name="w0")
                nc.vector.tensor_scalar(out=w0[:], in0=off[:], scalar1=0.0,
                                        scalar2=None, op0=Alu.is_ge)
                w1m = radix.tile([P, NT], F32, name="w1m")
                nc.vector.tensor_scalar(out=w1m[:], in0=off[:], scalar1=float(SEL),
                                        scalar2=None, op0=Alu.is_lt)
                m2 = radix.tile([P, NT], F32, name="m2")
                nc.vector.tensor_tensor(out=m2[:], in0=w0[:], in1=w1m[:], op=Alu.mult)
                m3 = radix.tile([P, NT], F32, name="m3")
                nc.vector.tensor_tensor(out=m3[:], in0=m2[:], in1=maskf[:], op=Alu.mult)
                t1 = radix.tile([P, NT], F32, name="t1")
                nc.vector.tensor_scalar(out=t1[:], in0=off[:],
                                        scalar1=-float(OOB_SENTINEL),
                                        scalar2=None, op0=Alu.add)
                t2 = radix.tile([P, NT], F32, name="t2")
                nc.vector.tensor_tensor(out=t2[:], in0=t1[:], in1=m3[:], op=Alu.mult)
                offf = radix.tile([P, NT], F32, name="offf")
                nc.vector.tensor_scalar(out=offf[:], in0=t2[:],
                                        scalar1=float(OOB_SENTINEL),
                                        scalar2=None, op0=Alu.add)

                # ---- rank -> token-id inversion (factored fp16 one-hot) ---------
                # H[p,c,j] = (128j <= rank < 128j+128); rm = rank mod 128.
                # Per column: lhsT S_lo[q,p'] = (rm[q,c] == p'), rhs R1 = low
                # token bits * H, R2 = H (hi bit). psum out1[p',j] + 2048*out2
                # = token id of rank slot j*128+p'. All values exact in fp16.
                offr = offf[:, :].to_broadcast([P, NT, NSJ])
                t1h = radix.tile([P, NT, NSJ], F32, name="t1h")
                nc.vector.tensor_tensor(out=t1h[:], in0=offr, in1=j128a_sb[:],
                                        op=Alu.is_ge)
                t2h = radix.tile([P, NT, NSJ], F32, name="t2h")
                nc.vector.tensor_tensor(out=t2h[:], in0=offr, in1=j128b_sb[:],
                                        op=Alu.is_lt)
                Hh = radix.tile([P, NT, NSJ], F32, name="Hh")
                nc.vector.tensor_tensor(out=Hh[:], in0=t1h[:], in1=t2h[:],
                                        op=Alu.mult)
                hj = radix.tile([P, NT, NSJ], F32, name="hj")
                nc.vector.tensor_tensor(out=hj[:], in0=Hh[:], in1=jvals_sb[:],
                                        op=Alu.mult)
                hidx = radix.tile([P, NT], F32, name="hidx")
                nc.vector.tensor_reduce(out=hidx[:], in_=hj[:],
                                        axis=mybir.AxisListType.X, op=Alu.add)
                rmt = radix.tile([P, NT], F32, name="rmt")
                nc.vector.tensor_scalar(out=rmt[:], in0=hidx[:], scalar1=-128.0,
                                        scalar2=None, op0=Alu.mult)
                rm2 = radix.tile([P, NT], F32, name="rm2")
                nc.vector.tensor_tensor(out=rm2[:], in0=rmt[:], in1=offf[:],
                                        op=Alu.add)
                lowr = lowf_sb[:, :].to_broadcast([P, NT, NSJ])
                R1 = radix.tile([P, NT, NSJ], FP16, name="R1")
                nc.vector.tensor_tensor(out=R1[:], in0=Hh[:], in1=lowr,
                                        op=Alu.mult)
                R2 = radix.tile([P, NT // 2, NSJ], FP16, name="R2")
                nc.vector.tensor_copy(out=R2[:], in_=Hh[:, NT // 2:, :])

                o1_ps = rx_psum.tile([P, NSJ], F32, name="o1_ps")
                o2_ps = rx_psum.tile([P, NSJ], F32, name="o2_ps")
                for c in range(NT):
                    slo = rjunk.tile([P, P], FP16, name="slo")
                    nc.vector.tensor_scalar(out=slo[:], in0=i128h_sb[:],
                                            scalar1=rm2[:, c:c + 1], scalar2=None,
                                            op0=Alu.is_equal)
                    nc.tensor.matmul(out=o1_ps[:], lhsT=slo[:], rhs=R1[:, c, :],
                                     start=(c == 0), stop=(c == NT - 1),
                                     skip_group_check=True)
                    if c >= NT // 2:
                        nc.tensor.matmul(out=o2_ps[:], lhsT=slo[:],
                                         rhs=R2[:, c - NT // 2, :],
                                         start=(c == NT // 2), stop=(c == NT - 1),
                                         skip_group_check=True)
                a2 = radix.tile([P, NSJ], F32, name="a2")
                nc.vector.tensor_copy(out=a2[:], in_=o1_ps[:])
                b2v = radix.tile([P, NSJ], F32, name="b2v")
                nc.vector.tensor_scalar(out=b2v[:], in0=o2_ps[:], scalar1=2048.0,
                                        scalar2=None, op0=Alu.mult)
                selff = radix.tile([P, NSJ], F32, name="selff")
                nc.vector.tensor_tensor(out=selff[:], in0=a2[:], in1=b2v[:],
                                        op=Alu.add)
                nc.vector.tensor_copy(out=selidx_sb[:], in_=selff[:])
                # host-visible token ids; not on the gather critical path
                nc.sync.dma_start(
                    out=sel_d.rearrange("(j p) one -> p (j one)", p=P),
                    in_=selff[:])

        # ---- gather (bf16 cast in DMA) + transpose + MLP -----------------------
        with ExitStack() as SM:
            ht_pool = SM.enter_context(tc.tile_pool(name="ht", bufs=1))
            xt_pool = SM.enter_context(tc.tile_pool(name="xt", bufs=1))
            ht = ht_pool.tile([P, NM, SEL], BF16)
            xt_all = xt_pool.tile([P, ND, SEL], BF16)

            with ExitStack() as SB:
                xsel_pool = SB.enter_context(tc.tile_pool(name="xsel", bufs=4))
                tp_psum = SB.enter_context(tc.tile_pool(name="tp_psum", bufs=2, space="PSUM"))
                for j in range(NSJ):
                    xs = xsel_pool.tile([P, D], BF16, name="xsel")
                    nc.gpsimd.indirect_dma_start(
                        out=xs[:], out_offset=None, in_=x_row,
                        in_offset=IndirectOffsetOnAxis(ap=selidx_sb[:, j:j + 1],
                                                       axis=0))
                    tpbig = tp_psum.tile([P, ND, P], BF16, name="tpbig")
                    for kd in range(ND):
                        nc.tensor.transpose(out=tpbig[:, kd, :],
                                            in_=xs[:, kd * P:(kd + 1) * P],
                                            identity=identb_sb[:])
                    nc.vector.tensor_copy(out=xt_all[:, :, j * P:(j + 1) * P],
                                          in_=tpbig[:, :, :])

            # ---- mm1: ht[m, tok] = gelu(w1^T x_sel^T + b1) ---------------------
            # n outer: the first token half only needs gather blocks j=0..3
            with ExitStack() as S1:
                mm1_psum = S1.enter_context(tc.tile_pool(name="mm1_psum", bufs=6, space="PSUM"))
                for n in range(2):
                    for m in range(NM):
                        ph = mm1_psum.tile([P, 512], F32, name="ph")
                        for kd in range(ND):
                            nc.tensor.matmul(
                                out=ph[:],
                                lhsT=w1bf[kd][:, m * P:(m + 1) * P],
                                rhs=xt_all[:, kd, n * 512:(n + 1) * 512],
                                start=(kd == 0), stop=(kd == ND - 1),
                            )
                        nc.scalar.activation(
                            out=ht[:, m, n * 512:(n + 1) * 512], in_=ph[:],
                            func=Act.Gelu_apprx_tanh, bias=b1t_sb[:, m:m + 1],
                            scale=1.0,
                        )

            # ---- mm2: y^T[d, tok] = w2^T ht + b2, stationary w2 chunks ---------
            with ExitStack() as SY:
                y_pool = SY.enter_context(tc.tile_pool(name="y", bufs=2))
                w2_pool = SY.enter_context(tc.tile_pool(name="w2s", bufs=5))
                mm2_psum = SY.enter_context(tc.tile_pool(name="mm2_psum", bufs=4, space="PSUM"))
                NDG = 4                      # d-groups of 2*P columns
                DCW = D // NDG               # 256
                for dg in range(NDG):
                    pz = [[mm2_psum.tile([P, 512], F32, name="pz") for _ in range(2)]
                          for _ in range(2)]
                    for kg in range(NM // NKGRP):
                        w2t = w2_pool.tile([P, NKGRP, DCW], BF16, name="w2t")
                        src = w2b.rearrange("(g p) f -> p g f", p=P)[
                            :, kg * NKGRP:(kg + 1) * NKGRP,
                            dg * DCW:(dg + 1) * DCW]
                        nc.gpsimd.dma_start(out=w2t[:], in_=src)
                        for ki in range(NKGRP):
                            kk = kg * NKGRP + ki
                            for dc in range(2):
                                for n in range(2):
                                    nc.tensor.matmul(
                                        out=pz[dc][n][:],
                                        lhsT=w2t[:, ki, dc * P:(dc + 1) * P],
                                        rhs=ht[:, kk, n * 512:(n + 1) * 512],
                                        start=(kk == 0), stop=(kk == NM - 1),
                                        skip_group_check=True,
                                    )
                    for dc in range(2):
                        dd = dg * 2 + dc
                        ysb = y_pool.tile([P, SEL], BF16, name="ysb")
                        nc.scalar.activation(
                            out=ysb[:, 0:512], in_=pz[dc][0][:],
                            func=Act.Identity,
                            bias=b2t_sb[:, dd:dd + 1], scale=1.0)
                        nc.vector.tensor_scalar(
                            out=ysb[:, 512:1024], in0=pz[dc][1][:],
                            scalar1=b2t_sb[:, dd:dd + 1], scalar2=None,
                            op0=Alu.add)
                        nc.sync.dma_start(
                            out=y_d.rearrange("(g p) s -> p g s", p=P)[:, dd, :],
                            in_=ysb[:])

    nc.compile()
    return nc


def make_consts():
    import ml_dtypes
    q = np.arange(P)
    j = np.arange(NSJ)
    c = np.arange(NT)
    j128a = np.broadcast_to(128.0 * j, (P, NT, NSJ)).astype(np.float32)
    jvals = np.broadcast_to(1.0 * j, (P, NT, NSJ)).astype(np.float32)
    tok = (c[None, :] * P + q[:, None])
    return {
        "j128a": j128a,
        "j128b": j128a + 128.0,
        "jvals": jvals,
        "lowf": (tok % 2048).astype(np.float32),
        "i128h": np.broadcast_to(q.astype(np.float16), (P, P)).copy(),
        "ident128": np.eye(P, dtype=np.float32),
        "identb128": np.eye(P, dtype=ml_dtypes.bfloat16),
        "ltri128": (q[:, None] < q[None, :]).astype(np.float32),  # [q, p] = q < p
        "slt32": (np.arange(NT)[:, None] < np.arange(NT)[None, :]).astype(np.float32),
        "id32": np.eye(NT, dtype=np.float32),
        "ones_1x128": np.ones((1, P), np.float32),
        "ones_128x1": np.ones((P, 1), np.float32),
        "ones_32x128": np.ones((NT, P), np.float32),
    }


def make_in_maps(x, W1, b1, W2, b2, wr, br):
    import ml_dtypes
    consts = make_consts()
    x = np.ascontiguousarray(np.asarray(x, np.float32))
    w1b = np.asarray(W1, np.float32).astype(ml_dtypes.bfloat16)
    w2b = np.asarray(W2, np.float32).astype(ml_dtypes.bfloat16)
    in_maps = []
    for c in range(NCORES):
        b, h = divmod(c, 2)
        m = {
            "x_row": x[b],
            "w1b": w1b,
            "w2b": w2b,
            "wr": np.asarray(wr, np.float32).reshape(1, D),
            "b1t": np.ascontiguousarray(np.asarray(b1, np.float32).reshape(NM, P).T),
            "b2t": np.ascontiguousarray(np.asarray(b2, np.float32).reshape(ND, P).T),
            "hbase": np.array([[h * SEL]], np.float32),
        }
        m.update(consts)
        in_maps.append(m)
    return in_maps


_NC_CACHE = None


def _get_program():
    global _NC_CACHE
    if _NC_CACHE is None:
        _NC_CACHE = build_program()
    return _NC_CACHE


def kernel(x, W1, b1, W2, b2, wr, br):
    from concourse.bass_utils import run_bass_kernel_spmd

    nc = _get_program()
    in_maps = make_in_maps(x, W1, b1, W2, b2, wr, br)
    res = run_bass_kernel_spmd(nc, in_maps, list(range(NCORES))).results
    out = np.zeros((B, L, D), np.float32)
    for c in range(NCORES):
        b, _h = divmod(c, 2)
        idx = np.asarray(res[c]["sel_d"]).reshape(SEL).astype(np.int64)
        y = np.asarray(res[c]["y_d"]).astype(np.float32)    # [D, SEL]
        out[b, idx] = y.T
    return out


# revision 29
# speedup vs baseline: 2.6853x; 1.0051x over previous
"""MoD (mixture-of-depths) MLP wrapper kernel for Trainium2, 8 NeuronCores.

Sharding: core c handles batch row b = c//2 and the half of that row's
top-K tokens with global selection ranks in [h*1024, (h+1)*1024), h = c%2.
Each core computes the full row's router scores + top-K threshold locally
(no collectives), inverts rank->token via an fp16 one-hot compaction,
gathers its 1024 token rows (bf16 cast in DMA), runs the FFN in bf16
(fp32 accumulation), and writes a compact result + the token ids.
The host places rows at their token positions while unsharding.

y is produced transposed ([D, SEL]) so mm2 can reuse stationary weights
across the full token width and fuse the output bias per-partition.
"""

import sys

sys.path.insert(0, "/opt/trn_rl_repo")

from contextlib import ExitStack

import numpy as np

from concourse import bass, bass_isa, mybir
from concourse import bacc
import concourse.tile as tile
from concourse.bass import IndirectOffsetOnAxis

B, L, D = 4, 4096, 1024
DFF = 4 * D
K = L // 2              # 2048 selected tokens per row
NCORES = 8
P = 128
NT = L // P             # 32 token tiles per row
SEL = K // 2            # 1024 selected tokens per core
NSJ = SEL // P          # 8 selected-token blocks
ND = D // P             # 8 d chunks
NM = DFF // P           # 32 dff tiles
NKGRP = 4               # w2 k-chunks per streamed tile
RADIX_PASSES = 3
OOB_SENTINEL = 2 * L

F32 = mybir.dt.float32
BF16 = mybir.dt.bfloat16
FP16 = mybir.dt.float16
I32 = mybir.dt.int32
Alu = mybir.AluOpType
Act = mybir.ActivationFunctionType


def build_program():
    nc = bacc.Bacc(
        "TRN2",
        target_bir_lowering=False,
        debug=False,
        enable_asserts=False,
        num_devices=NCORES,
    )

    x_row = nc.dram_tensor("x_row", [L, D], F32, kind="ExternalInput").ap()
    w1b = nc.dram_tensor("w1b", [D, DFF], BF16, kind="ExternalInput").ap()
    w2b = nc.dram_tensor("w2b", [DFF, D], BF16, kind="ExternalInput").ap()
    wrb_h = nc.dram_tensor("wrb_h", [P, D], F32, kind="ExternalInput").ap()
    b1t = nc.dram_tensor("b1t", [P, NM], F32, kind="ExternalInput").ap()
    b2t = nc.dram_tensor("b2t", [P, ND], F32, kind="ExternalInput").ap()
    hbase = nc.dram_tensor("hbase", [1, 1], F32, kind="ExternalInput").ap()
    ident = nc.dram_tensor("ident128", [P, P], F32, kind="ExternalInput").ap()
    identb = nc.dram_tensor("identb128", [P, P], BF16, kind="ExternalInput").ap()
    ltri = nc.dram_tensor("ltri128", [P, P], F32, kind="ExternalInput").ap()
    slt32 = nc.dram_tensor("slt32", [NT, NT], F32, kind="ExternalInput").ap()
    id32 = nc.dram_tensor("id32", [NT, NT], F32, kind="ExternalInput").ap()
    ones_1x128 = nc.dram_tensor("ones_1x128", [1, P], F32, kind="ExternalInput").ap()
    ones_128x1 = nc.dram_tensor("ones_128x1", [P, 1], F32, kind="ExternalInput").ap()
    ones_32x128 = nc.dram_tensor("ones_32x128", [NT, P], F32, kind="ExternalInput").ap()
    j128a = nc.dram_tensor("j128a", [P, NT, NSJ], F32, kind="ExternalInput").ap()
    j128b = nc.dram_tensor("j128b", [P, NT, NSJ], F32, kind="ExternalInput").ap()
    jvals = nc.dram_tensor("jvals", [P, NT, NSJ], F32, kind="ExternalInput").ap()
    lowf = nc.dram_tensor("lowf", [P, NT], F32, kind="ExternalInput").ap()
    i128h = nc.dram_tensor("i128h", [P, P], FP16, kind="ExternalInput").ap()

    y_d = nc.dram_tensor("y_d", [D, SEL], BF16, kind="ExternalOutput").ap()
    sel_d = nc.dram_tensor("sel_d", [SEL, 1], F32, kind="ExternalOutput").ap()

    with tile.TileContext(nc) as tc, ExitStack() as S0:
        const = S0.enter_context(tc.tile_pool(name="const", bufs=1))
        w1_pool = S0.enter_context(tc.tile_pool(name="w1bf", bufs=1))

        def cload(pool, ap, shape, dtype=F32, name=None):
            t = pool.tile(shape, dtype, name=name)
            nc.sync.dma_start(out=t[:], in_=ap)
            return t

        # replicated router weights lead the sync queue, ahead of the x tiles
        wrb = const.tile([P, D], F32, name="c_wrb")
        nc.sync.dma_start(out=wrb[:], in_=wrb_h)
        o1x128_sb = const.tile([1, P], F32, name="c_o1")
        nc.gpsimd.dma_start(out=o1x128_sb[:], in_=ones_1x128)
        ident_sb = const.tile([P, P], F32, name="c_id")
        nc.scalar.dma_start(out=ident_sb[:], in_=ident)

        iota_i = const.tile([P, 1], I32)
        nc.gpsimd.iota(iota_i[:], pattern=[[1, 1]], base=0, channel_multiplier=1)
        iota_f = const.tile([P, 1], F32)
        nc.vector.tensor_copy(out=iota_f[:], in_=iota_i[:])


        scores_sb = const.tile([P, NT], F32)
        selidx_sb = const.tile([P, NSJ], I32)

        with ExitStack() as SREP:
            rep_pool = SREP.enter_context(tc.tile_pool(name="rep", bufs=1))
            scores_row = rep_pool.tile([1, L], F32)
            scores_rep = rep_pool.tile([P, L], F32)

            # ---- phase A: router scores (fp32, exact) + replicated scores ------
            with ExitStack() as SA:
                apool = SA.enter_context(tc.tile_pool(name="apool", bufs=1))
                xs_pool = SA.enter_context(tc.tile_pool(name="xs", bufs=8))
                junk_pool = SA.enter_context(tc.tile_pool(name="junk", bufs=2))
                pa_psum = SA.enter_context(tc.tile_pool(name="pa_psum", bufs=2, space="PSUM"))

                PSP = 640        # DVE's share of the score product
                for t in range(NT):
                    x_t = xs_pool.tile([P, D], F32)
                    nc.sync.dma_start(out=x_t[:], in_=x_row[t * P:(t + 1) * P, :])
                    prod = junk_pool.tile([P, D], F32, name="prod")
                    nc.vector.tensor_tensor(out=prod[:, :PSP], in0=x_t[:, :PSP],
                                            in1=wrb[:, :PSP], op=Alu.mult)
                    nc.gpsimd.tensor_tensor(out=prod[:, PSP:], in0=x_t[:, PSP:],
                                            in1=wrb[:, PSP:], op=Alu.mult)
                    sink = junk_pool.tile([P, D], BF16, name="sink")
                    nc.scalar.activation(out=sink[:], in_=prod[:], func=Act.Identity,
                                         bias=0.0, scale=1.0,
                                         accum_out=scores_sb[:, t:t + 1])
                    tpp = pa_psum.tile([1, P], F32, name="pa_tp")
                    nc.tensor.transpose(out=tpp[:], in_=scores_sb[:, t:t + 1],
                                        identity=ident_sb[:])
                    nc.vector.tensor_copy(out=scores_row[:, t * P:(t + 1) * P],
                                          in_=tpp[:])
                    if t % 4 == 3:
                        n = t // 4
                        bp = pa_psum.tile([P, 512], F32, name="pa_mp")
                        nc.tensor.matmul(out=bp[:], lhsT=o1x128_sb[:],
                                         rhs=scores_row[:, n * 512:(n + 1) * 512],
                                         start=True, stop=True)
                        nc.vector.tensor_copy(
                            out=scores_rep[:, n * 512:(n + 1) * 512], in_=bp[:])

            # ---- deferred consts + resident w1 (sync queue, after x) -----------
            b1t_sb = cload(const, b1t, [P, NM], name="c_b1t")
            b2t_sb = cload(const, b2t, [P, ND], name="c_b2t")
            hb_sb = cload(const, hbase, [1, 1], name="c_hb")
            identb_sb = cload(const, identb, [P, P], BF16, name="c_idb")
            ltri_sb = cload(const, ltri, [P, P], name="c_lt")
            slt32_sb = cload(const, slt32, [NT, NT], name="c_sl")
            id32_sb = cload(const, id32, [NT, NT], name="c_id32")
            o128x1_sb = cload(const, ones_128x1, [P, 1], name="c_oc")
            o32x128_sb = cload(const, ones_32x128, [NT, P], name="c_o32")
            j128a_sb = cload(const, j128a, [P, NT, NSJ], name="c_j128a")
            j128b_sb = cload(const, j128b, [P, NT, NSJ], name="c_j128b")
            jvals_sb = cload(const, jvals, [P, NT, NSJ], name="c_jvals")
            lowf_sb = cload(const, lowf, [P, NT], name="c_lowf")
            i128h_sb = cload(const, i128h, [P, P], FP16, name="c_i128h")
            hb_col = const.tile([P, 1], F32)
            nc.gpsimd.partition_broadcast(hb_col[:], hb_sb[:])

            w1bf = []
            for kd in range(ND):
                t_ = w1_pool.tile([P, DFF], BF16, name=f"w1bf_{kd}")
                nc.sync.dma_start(out=t_[:], in_=w1b[kd * P:(kd + 1) * P, :])
                w1bf.append(t_)

            # ---- phase C: top-K threshold, 128-way bisection, Act+DVE split ----
            with ExitStack() as SC:
                radix = SC.enter_context(tc.tile_pool(name="radix", bufs=2))
                rjunk = SC.enter_context(tc.tile_pool(name="rjunk", bufs=2))
                rx_psum = SC.enter_context(tc.tile_pool(name="rx_psum", bufs=1, space="PSUM"))

                ACOLS = 2624     # Act's share of the count scan
                DCOLS = L - ACOLS
                neglo = radix.tile([P, 1], F32, name="neglo")
                nc.vector.memset(neglo[:], 16.0)
                w_cur = 32.0 / P
                for _pass in range(RADIX_PASSES):
                    negthr = radix.tile([P, 1], F32, name="negthr")
                    nc.vector.tensor_scalar(out=negthr[:], in0=iota_f[:],
                                            scalar1=-w_cur, scalar2=neglo[:],
                                            op0=Alu.mult, op1=Alu.add)
                    # Act share: acc1 = sum sign(score - thr) = 2*c1 - ACOLS
                    acc1 = radix.tile([P, 1], F32, name="acc1")
                    sink2 = rjunk.tile([P, ACOLS], BF16, name="sink2")
                    nc.scalar.activation(out=sink2[:], in_=scores_rep[:, :ACOLS],
                                         func=Act.Sign, bias=negthr[:], scale=1.0,
                                         accum_out=acc1[:])
                    # DVE share: indicator then 2x bf16 reduce -> c2
                    c2 = radix.tile([P, 1], F32, name="c2")
                    sinkd = rjunk.tile([P, DCOLS], BF16, name="sinkd")
                    nc.vector.tensor_scalar(out=sinkd[:], in0=scores_rep[:, ACOLS:],
                                            scalar1=negthr[:], scalar2=0.0,
                                            op0=Alu.add, op1=Alu.is_ge)
                    nc.vector.tensor_reduce(out=c2[:], in_=sinkd[:],
                                            axis=mybir.AxisListType.X, op=Alu.add)
                    # count >= K  <=>  acc1 + 2*c2 >= 2K - ACOLS
                    comb = radix.tile([P, 1], F32, name="comb")
                    nc.vector.tensor_scalar(out=comb[:], in0=c2[:], scalar1=2.0,
                                            scalar2=acc1[:], op0=Alu.mult,
                                            op1=Alu.add)
                    sel = radix.tile([P, 1], F32, name="sel")
                    nc.vector.tensor_scalar(out=sel[:], in0=comb[:],
                                            scalar1=float(2 * K - ACOLS),
                                            scalar2=None, op0=Alu.is_ge)
                    s_col = radix.tile([P, 1], F32, name="s_col")
                    nc.gpsimd.partition_all_reduce(s_col[:], sel[:], channels=P,
                                                   reduce_op=bass_isa.ReduceOp.add)
                    delta = radix.tile([P, 1], F32, name="delta")
                    nc.vector.tensor_scalar(out=delta[:], in0=s_col[:],
                                            scalar1=-w_cur, scalar2=w_cur,
                                            op0=Alu.mult, op1=Alu.add)
                    neglo2 = radix.tile([P, 1], F32, name="neglo")
                    nc.vector.tensor_tensor(out=neglo2[:], in0=neglo[:],
                                            in1=delta[:], op=Alu.add)
                    neglo = neglo2
                    w_cur /= P

                T_col = radix.tile([P, 1], F32, name="T_col")
                nc.vector.tensor_scalar(out=T_col[:], in0=neglo[:], scalar1=-1.0,
                                        scalar2=None, op0=Alu.mult)

                # ---- mask, global rank, local window offsets --------------------
                maskf = radix.tile([P, NT], F32, name="maskf")
                nc.vector.tensor_scalar(out=maskf[:], in0=scores_sb[:],
                                        scalar1=T_col[:], scalar2=None,
                                        op0=Alu.is_ge)
                colsum_p = rx_psum.tile([NT, 1], F32, name="cs_ps")
                nc.tensor.matmul(out=colsum_p[:], lhsT=maskf[:], rhs=o128x1_sb[:],
                                 start=True, stop=True)
                colsum = radix.tile([NT, 1], F32, name="colsum")
                nc.vector.tensor_copy(out=colsum[:], in_=colsum_p[:])
                excl_p = rx_psum.tile([NT, 1], F32, name="ex_ps")
                nc.tensor.matmul(out=excl_p[:], lhsT=slt32_sb[:], rhs=colsum[:],
                                 start=True, stop=True)
                excl = radix.tile([NT, 1], F32, name="excl")
                nc.vector.tensor_copy(out=excl[:], in_=excl_p[:])
                diag = radix.tile([NT, NT], F32, name="diag")
                nc.vector.tensor_tensor(out=diag[:], in0=id32_sb[:],
                                        in1=excl[:, :1].to_broadcast([NT, NT]),
                                        op=Alu.mult)
                rank_p = rx_psum.tile([P, NT], F32, name="rank_ps")
                nc.tensor.matmul(out=rank_p[:], lhsT=ltri_sb[:], rhs=maskf[:],
                                 start=True, stop=False, skip_group_check=True)
                nc.tensor.matmul(out=rank_p[:], lhsT=o32x128_sb[:], rhs=diag[:],
                                 start=False, stop=True, skip_group_check=True)

                off = radix.tile([P, NT], F32, name="off")
                nc.vector.tensor_scalar(out=off[:], in0=rank_p[:],
                                        scalar1=hb_col[:], scalar2=None,
                                        op0=Alu.subtract)
                w0 = radix.tile([P, NT], F32, name="w0")
                nc.vector.tensor_scalar(out=w0[:], in0=off[:], scalar1=0.0,
                                        scalar2=None, op0=Alu.is_ge)
                w1m = radix.tile([P, NT], F32, name="w1m")
                nc.vector.tensor_scalar(out=w1m[:], in0=off[:], scalar1=float(SEL),
                                        scalar2=None, op0=Alu.is_lt)
                m2 = radix.tile([P, NT], F32, name="m2")
                nc.vector.tensor_tensor(out=m2[:], in0=w0[:], in1=w1m[:], op=Alu.mult)
                m3 = radix.tile([P, NT], F32, name="m3")
                nc.vector.tensor_tensor(out=m3[:], in0=m2[:], in1=maskf[:], op=Alu.mult)
                t1 = radix.tile([P, NT], F32, name="t1")
                nc.vector.tensor_scalar(out=t1[:], in0=off[:],
                                        scalar1=-float(OOB_SENTINEL),
                                        scalar2=None, op0=Alu.add)
                t2 = radix.tile([P, NT], F32, name="t2")
                nc.vector.tensor_tensor(out=t2[:], in0=t1[:], in1=m3[:], op=Alu.mult)
                offf = radix.tile([P, NT], F32, name="offf")
                nc.vector.tensor_scalar(out=offf[:], in0=t2[:],
                                        scalar1=float(OOB_SENTINEL),
                                        scalar2=None, op0=Alu.add)

                # ---- rank -> token-id inversion (factored fp16 one-hot) ---------
                # H[p,c,j] = (128j <= rank < 128j+128); rm = rank mod 128.
                # Per column: lhsT S_lo[q,p'] = (rm[q,c] == p'), rhs R1 = low
                # token bits * H, R2 = H (hi bit). psum out1[p',j] + 2048*out2
                # = token id of rank slot j*128+p'. All values exact in fp16.
                offr = offf[:, :].to_broadcast([P, NT, NSJ])
                t1h = radix.tile([P, NT, NSJ], F32, name="t1h")
                nc.vector.tensor_tensor(out=t1h[:], in0=offr, in1=j128a_sb[:],
                                        op=Alu.is_ge)
                t2h = radix.tile([P, NT, NSJ], F32, name="t2h")
                nc.vector.tensor_tensor(out=t2h[:], in0=offr, in1=j128b_sb[:],
                                        op=Alu.is_lt)
                Hh = radix.tile([P, NT, NSJ], F32, name="Hh")
                nc.vector.tensor_tensor(out=Hh[:], in0=t1h[:], in1=t2h[:],
                                        op=Alu.mult)
                hj = radix.tile([P, NT, NSJ], F32, name="hj")
                nc.vector.tensor_tensor(out=hj[:], in0=Hh[:], in1=jvals_sb[:],
                                        op=Alu.mult)
                hidx = radix.tile([P, NT], F32, name="hidx")
                nc.vector.tensor_reduce(out=hidx[:], in_=hj[:],
                                        axis=mybir.AxisListType.X, op=Alu.add)
                rmt = radix.tile([P, NT], F32, name="rmt")
                nc.vector.tensor_scalar(out=rmt[:], in0=hidx[:], scalar1=-128.0,
                                        scalar2=None, op0=Alu.mult)
                rm2 = radix.tile([P, NT], F32, name="rm2")
                nc.vector.tensor_tensor(out=rm2[:], in0=rmt[:], in1=offf[:],
                                        op=Alu.add)
                lowr = lowf_sb[:, :].to_broadcast([P, NT, NSJ])
                R1 = radix.tile([P, NT, NSJ], FP16, name="R1")
                nc.vector.tensor_tensor(out=R1[:], in0=Hh[:], in1=lowr,
                                        op=Alu.mult)
                R2 = radix.tile([P, NT // 2, NSJ], FP16, name="R2")
                nc.vector.tensor_copy(out=R2[:], in_=Hh[:, NT // 2:, :])

                o1_ps = rx_psum.tile([P, NSJ], F32, name="o1_ps")
                o2_ps = rx_psum.tile([P, NSJ], F32, name="o2_ps")
                for c in range(NT):
                    slo = rjunk.tile([P, P], FP16, name="slo")
                    nc.vector.tensor_scalar(out=slo[:], in0=i128h_sb[:],
                                            scalar1=rm2[:, c:c + 1], scalar2=None,
                                            op0=Alu.is_equal)
                    nc.tensor.matmul(out=o1_ps[:], lhsT=slo[:], rhs=R1[:, c, :],
                                     start=(c == 0), stop=(c == NT - 1),
                                     skip_group_check=True)
                    if c >= NT // 2:
                        nc.tensor.matmul(out=o2_ps[:], lhsT=slo[:],
                                         rhs=R2[:, c - NT // 2, :],
                                         start=(c == NT // 2), stop=(c == NT - 1),
                                         skip_group_check=True)
                a2 = radix.tile([P, NSJ], F32, name="a2")
                nc.vector.tensor_copy(out=a2[:], in_=o1_ps[:])
                b2v = radix.tile([P, NSJ], F32, name="b2v")
                nc.vector.tensor_scalar(out=b2v[:], in0=o2_ps[:], scalar1=2048.0,
                                        scalar2=None, op0=Alu.mult)
                selff = radix.tile([P, NSJ], F32, name="selff")
                nc.vector.tensor_tensor(out=selff[:], in0=a2[:], in1=b2v[:],
                                        op=Alu.add)
                nc.vector.tensor_copy(out=selidx_sb[:], in_=selff[:])
                # host-visible token ids; not on the gather critical path
                nc.sync.dma_start(
                    out=sel_d.rearrange("(j p) one -> p (j one)", p=P),
                    in_=selff[:])

        # ---- gather (bf16 cast in DMA) + transpose + MLP -----------------------
        with ExitStack() as SM:
            ht_pool = SM.enter_context(tc.tile_pool(name="ht", bufs=1))
            xt_pool = SM.enter_context(tc.tile_pool(name="xt", bufs=1))
            ht = ht_pool.tile([P, NM, SEL], BF16)
            xt_all = xt_pool.tile([P, ND, SEL], BF16)

            with ExitStack() as SB:
                xsel_pool = SB.enter_context(tc.tile_pool(name="xsel", bufs=4))
                tp_psum = SB.enter_context(tc.tile_pool(name="tp_psum", bufs=2, space="PSUM"))
                for j in range(NSJ):
                    xs = xsel_pool.tile([P, D], BF16, name="xsel")
                    nc.gpsimd.indirect_dma_start(
                        out=xs[:], out_offset=None, in_=x_row,
                        in_offset=IndirectOffsetOnAxis(ap=selidx_sb[:, j:j + 1],
                                                       axis=0))
                    tpbig = tp_psum.tile([P, ND, P], BF16, name="tpbig")
                    for kd in range(ND):
                        nc.tensor.transpose(out=tpbig[:, kd, :],
                                            in_=xs[:, kd * P:(kd + 1) * P],
                                            identity=identb_sb[:])
                    nc.vector.tensor_copy(out=xt_all[:, :, j * P:(j + 1) * P],
                                          in_=tpbig[:, :, :])

            # ---- mm1: ht[m, tok] = gelu(w1^T x_sel^T + b1) ---------------------
            # n outer: the first token half only needs gather blocks j=0..3
            with ExitStack() as S1:
                mm1_psum = S1.enter_context(tc.tile_pool(name="mm1_psum", bufs=6, space="PSUM"))
                for n in range(2):
                    for m in range(NM):
                        ph = mm1_psum.tile([P, 512], F32, name="ph")
                        for kd in range(ND):
                            nc.tensor.matmul(
                                out=ph[:],
                                lhsT=w1bf[kd][:, m * P:(m + 1) * P],
                                rhs=xt_all[:, kd, n * 512:(n + 1) * 512],
                                start=(kd == 0), stop=(kd == ND - 1),
                            )
                        nc.scalar.activation(
                            out=ht[:, m, n * 512:(n + 1) * 512], in_=ph[:],
                            func=Act.Gelu_apprx_tanh, bias=b1t_sb[:, m:m + 1],
                            scale=1.0,
                        )

            # ---- mm2: y^T[d, tok] = w2^T ht + b2, stationary w2 chunks ---------
            with ExitStack() as SY:
                y_pool = SY.enter_context(tc.tile_pool(name="y", bufs=2))
                w2_pool = SY.enter_context(tc.tile_pool(name="w2s", bufs=5))
                mm2_psum = SY.enter_context(tc.tile_pool(name="mm2_psum", bufs=4, space="PSUM"))
                NDG = 4                      # d-groups of 2*P columns
                DCW = D // NDG               # 256
                for dg in range(NDG):
                    pz = [[mm2_psum.tile([P, 512], F32, name="pz") for _ in range(2)]
                          for _ in range(2)]
                    for kg in range(NM // NKGRP):
                        w2t = w2_pool.tile([P, NKGRP, DCW], BF16, name="w2t")
                        src = w2b.rearrange("(g p) f -> p g f", p=P)[
                            :, kg * NKGRP:(kg + 1) * NKGRP,
                            dg * DCW:(dg + 1) * DCW]
                        nc.gpsimd.dma_start(out=w2t[:], in_=src)
                        for ki in range(NKGRP):
                            kk = kg * NKGRP + ki
                            for dc in range(2):
                                for n in range(2):
                                    nc.tensor.matmul(
                                        out=pz[dc][n][:],
                                        lhsT=w2t[:, ki, dc * P:(dc + 1) * P],
                                        rhs=ht[:, kk, n * 512:(n + 1) * 512],
                                        start=(kk == 0), stop=(kk == NM - 1),
                                        skip_group_check=True,
                                    )
                    for dc in range(2):
                        dd = dg * 2 + dc
                        ysb = y_pool.tile([P, SEL], BF16, name="ysb")
                        nc.scalar.activation(
                            out=ysb[:, 0:512], in_=pz[dc][0][:],
                            func=Act.Identity,
                            bias=b2t_sb[:, dd:dd + 1], scale=1.0)
                        nc.vector.tensor_scalar(
                            out=ysb[:, 512:1024], in0=pz[dc][1][:],
                            scalar1=b2t_sb[:, dd:dd + 1], scalar2=None,
                            op0=Alu.add)
                        nc.sync.dma_start(
                            out=y_d.rearrange("(g p) s -> p g s", p=P)[:, dd, :],
                            in_=ysb[:])

    nc.compile()
    return nc


def make_consts():
    import ml_dtypes
    q = np.arange(P)
    j = np.arange(NSJ)
    c = np.arange(NT)
    j128a = np.broadcast_to(128.0 * j, (P, NT, NSJ)).astype(np.float32)
    jvals = np.broadcast_to(1.0 * j, (P, NT, NSJ)).astype(np.float32)
    tok = (c[None, :] * P + q[:, None])
    return {
        "j128a": j128a,
        "j128b": j128a + 128.0,
        "jvals": jvals,
        "lowf": (tok % 2048).astype(np.float32),
        "i128h": np.broadcast_to(q.astype(np.float16), (P, P)).copy(),
        "ident128": np.eye(P, dtype=np.float32),
        "identb128": np.eye(P, dtype=ml_dtypes.bfloat16),
        "ltri128": (q[:, None] < q[None, :]).astype(np.float32),  # [q, p] = q < p
        "slt32": (np.arange(NT)[:, None] < np.arange(NT)[None, :]).astype(np.float32),
        "id32": np.eye(NT, dtype=np.float32),
        "ones_1x128": np.ones((1, P), np.float32),
        "ones_128x1": np.ones((P, 1), np.float32),
        "ones_32x128": np.ones((NT, P), np.float32),
    }


def make_in_maps(x, W1, b1, W2, b2, wr, br):
    import ml_dtypes
    consts = make_consts()
    x = np.ascontiguousarray(np.asarray(x, np.float32))
    w1b = np.asarray(W1, np.float32).astype(ml_dtypes.bfloat16)
    w2b = np.asarray(W2, np.float32).astype(ml_dtypes.bfloat16)
    in_maps = []
    for c in range(NCORES):
        b, h = divmod(c, 2)
        m = {
            "x_row": x[b],
            "w1b": w1b,
            "w2b": w2b,
            "wrb_h": np.ascontiguousarray(
                np.broadcast_to(np.asarray(wr, np.float32).reshape(1, D), (P, D))),
            "b1t": np.ascontiguousarray(np.asarray(b1, np.float32).reshape(NM, P).T),
            "b2t": np.ascontiguousarray(np.asarray(b2, np.float32).reshape(ND, P).T),
            "hbase": np.array([[h * SEL]], np.float32),
        }
        m.update(consts)
        in_maps.append(m)
    return in_maps


_NC_CACHE = None


def _get_program():
    global _NC_CACHE
    if _NC_CACHE is None:
        _NC_CACHE = build_program()
    return _NC_CACHE


def kernel(x, W1, b1, W2, b2, wr, br):
    from concourse.bass_utils import run_bass_kernel_spmd

    nc = _get_program()
    in_maps = make_in_maps(x, W1, b1, W2, b2, wr, br)
    res = run_bass_kernel_spmd(nc, in_maps, list(range(NCORES))).results
    out = np.zeros((B, L, D), np.float32)
    for c in range(NCORES):
        b, _h = divmod(c, 2)
        idx = np.asarray(res[c]["sel_d"]).reshape(SEL).astype(np.int64)
        y = np.asarray(res[c]["y_d"]).astype(np.float32)    # [D, SEL]
        out[b, idx] = y.T
    return out
